# revision 36
# baseline (speedup 1.0000x reference)
"""Trainium2 Bass kernel for nn_GRIC_31550829756424 (GCN-attention block).

Data-parallel over batch: 8 batches -> 8 NeuronCores, one full batch per core.

Algebraic restructure vs the straightforward lowering:
  Q = adj @ (Hn Wq) = (adj Hn) Wq  ==>  G = adj_norm @ Hn computed ONCE.
  S^T_h = G M2_h G^T with M2_h = dk*Wk_h Wq_h^T folded on host, so scores need
  no Q/K materialization. Row-constant score-bias terms cancel in softmax and
  are dropped; the column term v = G @ (dk Wk_h b_Q,h) rides the exp bias.
  g1/be1 fold into W_O / a bias row; g2/be2 fold into W1/b1 for the MLP.
  B_bias is pre-transposed + fp8-cast on host and added into PSUM by the PE
  via identity matmuls (DoubleRow fp8: 0.5 cyc/row).
  LayerNorm rstd uses exp(-0.5*ln(x)) so the ACT engine never leaves the
  {exp, ln, relu} activation table (no table reloads).
  All transposes ride the DMA XBAR (A^T from DRAM, z->mhcT, oz->oT, r2T back).

Self-contained: hardcodes all shapes; imports only the in-container concourse
stack.
"""

import sys

sys.path.insert(0, "/opt/trn_rl_repo")

import numpy as np
import ml_dtypes
from contextlib import ExitStack

import concourse.bass as bass
import concourse.tile as tile
from concourse import bacc
from concourse import mybir
from concourse.bass_utils import run_bass_kernel_spmd

F32 = mybir.dt.float32
BF16 = mybir.dt.bfloat16
FP8 = mybir.dt.float8e4
AF = mybir.ActivationFunctionType
OP = mybir.AluOpType
AX = mybir.AxisListType
PM = mybir.MatmulPerfMode

B = 8
N = 1024
D = 128
HEADS = 8
DV = 128
HD = HEADS * DV  # 1024
P = 128
NT = N // P  # 8 tiles of 128 rows
DK = 1.0 / float(np.sqrt(np.float32(D)))
EPS = 1e-5

USE_DOUBLEROW = True
DEBUG_TAPS = False

_prog_cache = {}


def _bcast_load(nc, dst, src):
    """DMA-load 1D DRAM vector src [W] replicated across all P partitions of
    dst [P, W] (issued on the ACT HWDGE queue)."""
    rep = bass.AP(tensor=src.tensor, offset=src.offset, ap=[[0, P]] + list(src.ap))
    nc.scalar.dma_start(out=dst, in_=rep)


U32 = mybir.dt.uint32
_RSQRT_MAGIC = 0x5F3759DF


def _rsqrt_pool(nc, pool, out_ap, in_ap, w, tag, cns):
    """Pool-engine rsqrt via tensor_tensor-only ops (Pool rejects
    TensorScalarPtr). cns = dict of const tiles sliced to [P, w]:
    one (u32 1), magic, c15 (1.5), cm05 (-0.5)."""
    eng = nc.gpsimd
    y = pool.tile([P, w], F32, name=f"rp_y{tag}", tag=f"rp_y{w}")
    ib = pool.tile([P, w], U32, name=f"rp_i{tag}", tag=f"rp_i{w}")
    # seed on DVE (Pool shifts require u64 outputs); iterations on Pool
    nc.vector.tensor_scalar(
        out=ib, in0=in_ap.bitcast(U32), scalar1=1, scalar2=None,
        op0=OP.logical_shift_right)
    nc.vector.tensor_tensor(out=y.bitcast(U32), in0=cns["magic"], in1=ib,
                            op=OP.subtract)
    for it in range(2):
        a = pool.tile([P, w], F32, name=f"rp_a{tag}_{it}", tag=f"rp_a{w}")
        eng.tensor_tensor(out=a, in0=y, in1=y, op=OP.mult)
        eng.tensor_tensor(out=a, in0=a, in1=in_ap, op=OP.mult)
        eng.tensor_tensor(out=a, in0=a, in1=cns["cm05"], op=OP.mult)
        eng.tensor_tensor(out=a, in0=a, in1=cns["c15"], op=OP.add)
        dst = out_ap if it == 1 else y
        eng.tensor_tensor(out=dst, in0=y, in1=a, op=OP.mult)


def _rsqrt(nc, eng, pool, out_ap, in_ap, w, tag, magic):
    """out = 1/sqrt(in) elementwise on [P, w] f32 via bit-hack seed + 2 Newton
    iterations. Runs entirely on `eng` (vector or gpsimd) — avoids the ACT
    table thrash that Sqrt/Ln would cause next to Exp."""
    y = pool.tile([P, w], F32, name=f"rq_y{tag}", tag=f"rq_y{w}")
    ib = pool.tile([P, w], U32, name=f"rq_i{tag}", tag=f"rq_i{w}")
    eng.tensor_scalar(
        out=ib, in0=in_ap.bitcast(U32), scalar1=1, scalar2=None,
        op0=OP.logical_shift_right)
    eng.tensor_tensor(out=y.bitcast(U32), in0=magic, in1=ib, op=OP.subtract)
    for it in range(2):
        a = pool.tile([P, w], F32, name=f"rq_a{tag}_{it}", tag=f"rq_a{w}")
        eng.tensor_tensor(out=a, in0=y, in1=y, op=OP.mult)
        eng.tensor_tensor(out=a, in0=a, in1=in_ap, op=OP.mult)
        eng.tensor_scalar(
            out=a, in0=a, scalar1=-0.5, scalar2=1.5, op0=OP.mult, op1=OP.add)
        dst = out_ap if it == 1 else y
        eng.tensor_tensor(out=dst, in0=y, in1=a, op=OP.mult)


def _build_program():
    nc = bacc.Bacc(None)

    h_in = nc.declare_dram_parameter("h", [N, D], F32, isOutput=False)
    a16_in = nc.declare_dram_parameter("a16", [N, N], BF16, isOutput=False)
    bt_in = nc.declare_dram_parameter("bt", [HEADS, N, N], FP8, isOutput=False)
    wp32_in = nc.declare_dram_parameter("wp32", [P, 1666], F32, isOutput=False)
    wp16_in = nc.declare_dram_parameter("wp16", [P, 3464], BF16, isOutput=False)
    out_dram = nc.declare_dram_parameter("out", [N, D], F32, isOutput=True)
    taps = {}
    if DEBUG_TAPS:
        taps["t_disrow"] = nc.declare_dram_parameter(
            "t_disrow", [P, N], F32, isOutput=True)
        taps["t_GT"] = nc.declare_dram_parameter(
            "t_GT", [P, N], BF16, isOutput=True)
        taps["t_aT"] = nc.declare_dram_parameter(
            "t_aT", [P, NT, N], BF16, isOutput=True)
        taps["t_hnS"] = nc.declare_dram_parameter(
            "t_hnS", [P, NT, D], BF16, isOutput=True)
        taps["t_vna"] = nc.declare_dram_parameter(
            "t_vna", [P, NT, HEADS, DV + 1], BF16, isOutput=True)
        taps["t_et0"] = nc.declare_dram_parameter(
            "t_et0", [P, NT, N], BF16, isOutput=True)
        taps["t_mhcT"] = nc.declare_dram_parameter(
            "t_mhcT", [P, HEADS, N], BF16, isOutput=True)
        taps["t_orow"] = nc.declare_dram_parameter(
            "t_orow", [P, NT, D], F32, isOutput=True)
        taps["t_r2n"] = nc.declare_dram_parameter(
            "t_r2n", [P, NT, D], BF16, isOutput=True)
    disr = nc.dram_tensor("disr", [N], F32, kind="Internal")

    with tile.TileContext(nc) as tc, ExitStack() as ctx:
        consts = ctx.enter_context(tc.tile_pool(name="consts", bufs=1))
        persist = ctx.enter_context(tc.tile_pool(name="persist", bufs=1))
        small = ctx.enter_context(tc.tile_pool(name="small", bufs=12))

        # ---- constants -------------------------------------------------
        # diag-fix masks in bf16: omi = 1 - I, identb = I
        omi = consts.tile([P, P], BF16)
        nc.gpsimd.memset(omi, 1.0)
        nc.gpsimd.affine_select(
            out=omi, in_=omi, compare_op=OP.not_equal, fill=0.0,
            base=0, pattern=[[-1, P]], channel_multiplier=1)
        identb = consts.tile([P, P], BF16)
        nc.gpsimd.memset(identb, 0.0)
        nc.gpsimd.affine_select(
            out=identb, in_=identb, compare_op=OP.not_equal, fill=1.0,
            base=0, pattern=[[-1, P]], channel_multiplier=1)
        if USE_DOUBLEROW:
            # [I | 0] and [0 | I] fp8 stationaries for DoubleRow bias-adds
            id2a = consts.tile([P, 2, P], FP8)
            nc.gpsimd.memset(id2a, 0.0)
            nc.vector.tensor_copy(out=id2a[:, 0, :], in_=identb)
            id2b = consts.tile([P, 2, P], FP8)
            nc.gpsimd.memset(id2b, 0.0)
            nc.vector.tensor_copy(out=id2b[:, 1, :], in_=identb)
        else:
            id8 = consts.tile([P, P], FP8)
            nc.vector.tensor_copy(out=id8, in_=identb)
        ones1 = consts.tile([1, P], BF16)
        nc.vector.memset(ones1, 1.0)
        magic8 = consts.tile([P, NT], U32)
        nc.vector.memset(magic8, _RSQRT_MAGIC)
        one8 = consts.tile([P, NT], U32)
        nc.vector.memset(one8, 1)
        c158 = consts.tile([P, NT], F32)
        nc.vector.memset(c158, 1.5)
        cm058 = consts.tile([P, NT], F32)
        nc.vector.memset(cm058, -0.5)
        onescol = consts.tile([P, 1], BF16)
        nc.vector.memset(onescol, 1.0)

        # ---- phase A/B: H + A + packed-weight loads ---------------------
        h_sb = persist.tile([P, NT, D], F32, tag="h")
        nc.scalar.dma_start(out=h_sb, in_=h_in.rearrange("(t p) d -> p t d", p=P))
        wp32 = consts.tile([P, 1666], F32)
        nc.scalar.dma_start(out=wp32, in_=wp32_in[:, :])
        wp16 = consts.tile([P, 3464], BF16)
        nc.scalar.dma_start(out=wp16, in_=wp16_in[:, :])
        g0b = wp32[:, 0:128]
        be0b = wp32[:, 128:256]
        g2b = wp32[:, 256:384]
        g3b = wp32[:, 384:512]
        be23b = wp32[:, 512:640]
        bvb = wp32[:, 640:1664]
        b1p = wp32[:, 1664:1665]
        b2p = wp32[:, 1665:1666]
        mh_sb = wp16[:, 0:1024].rearrange("p (hh d) -> p hh d", hh=HEADS)
        wv_sb = wp16[:, 1024:2048]
        wo_sb = wp16[:, 2048:3072].rearrange("p (hh d) -> p hh d", hh=HEADS)
        w1_sb = wp16[:, 3072:3200]
        w2_sb = wp16[:, 3200:3328]
        qb_sb = wp16[:, 3328:3336]
        bo_row = wp16[0:1, 3336:3464]

        # A loads: natural chunks (scoped) feed DVE rowsums; A^T via XBAR
        # into 8 SEPARATE tiles (a shared tile serializes the XBARs against
        # the diag-fix RMWs through tile-level dep tracking).
        rs_all = small.tile([P, NT], F32, tag="rs_all")
        mvH = small.tile([P, NT, 2], F32, tag="mvH")
        anat_ctx = tc.tile_pool(name="anatp", bufs=1)
        anatp = anat_ctx.__enter__()
        anat = anatp.tile([P, NT, N], BF16, tag="anat")
        aTt = persist.tile([P, NT, N], BF16, tag="aTt")  # [m-chunk, n]
        aT = [aTt[:, i, :] for i in range(NT)]
        psT_ctx = tc.tile_pool(name="psT", bufs=4, space=bass.MemorySpace.PSUM)
        psT = psT_ctx.__enter__()
        for j in range(NT):
            an = anat[:, j, :]
            nc.sync.dma_start(out=an, in_=a16_in[j * P:(j + 1) * P, :])
            db = anat[:, j, j * P:(j + 1) * P]
            nc.gpsimd.tensor_tensor(out=db, in0=db, in1=omi, op=OP.mult)
            nc.gpsimd.tensor_tensor(out=db, in0=db, in1=identb, op=OP.add)
            nc.vector.reduce_sum(
                out=rs_all[:, j:j + 1], in_=an, axis=AX.X)
            # transpose the 8 blocks of this natural chunk on the (idle) PE;
            # one strided 512-wide copy per 4 blocks, alternating ACT/DVE
            for g in range(2):
                pt = psT.tile([P, 4, P], BF16, tag="pt", name=f"pt{j}_{g}")
                for ii in range(4):
                    i = g * 4 + ii
                    nc.tensor.transpose(
                        pt[:, ii, :], anat[:, j, i * P:(i + 1) * P], identb)
                dst = aTt[:, g * 4:(g + 1) * 4, j * P:(j + 1) * P]
                if (j + g) % 2 == 0:
                    nc.scalar.copy(out=dst, in_=pt)
                else:
                    nc.vector.tensor_copy(out=dst, in_=pt)
        psT_ctx.__exit__(None, None, None)
        for j in range(NT):
            # H LN stats (independent of A)
            s6 = small.tile([P, 6], F32, tag="s6h", name=f"s6h{j}")
            nc.vector.bn_stats(out=s6, in_=h_sb[:, j, :])
            nc.vector.bn_aggr(out=mvH[:, j, :], in_=s6)

        # dis = rsqrt(max(rowsum, 1))  [P, NT]
        dmax = small.tile([P, NT], F32, tag="dmax")
        nc.vector.tensor_scalar_max(out=dmax, in0=rs_all, scalar1=1.0)
        dis_sb = small.tile([P, NT], F32, tag="dis")
        _rsqrt(nc, nc.vector, small, dis_sb, dmax, NT, "dis", magic8)
        anat_ctx.__exit__(None, None, None)

        # HnS = dis * LN(H):  rstd2 = rstdH*dis
        tvH = small.tile([P, NT], F32, tag="tvH")
        nc.vector.tensor_scalar_add(
            out=tvH, in0=mvH[:, :, 1], scalar1=EPS)
        rstdH = small.tile([P, NT], F32, tag="rstdH")
        _rsqrt(nc, nc.vector, small, rstdH, tvH, NT, "H", magic8)
        rstd2 = small.tile([P, NT], F32, tag="rstd2")
        nc.vector.tensor_tensor(out=rstd2, in0=rstdH, in1=dis_sb, op=OP.mult)

        # disrow [P, N] broadcast of dis in n-order via DRAM round-trip
        nc.sync.dma_start(out=disr.rearrange("(t p) -> p t", p=P), in_=dis_sb)
        disrow = persist.tile([P, N], F32, tag="disrow")
        rep = bass.AP(
            tensor=disr[:].tensor, offset=disr[:].offset,
            ap=[[0, P]] + list(disr[:].ap))
        nc.sync.dma_start(out=disrow, in_=rep)

        hnS = persist.tile([P, NT, D], BF16, tag="hnS")
        for j in range(NT):
            zh = small.tile([P, D], F32, tag="zh")
            nc.vector.tensor_scalar(
                out=zh, in0=h_sb[:, j, :], scalar1=mvH[:, j, 0:1],
                scalar2=rstd2[:, j:j + 1], op0=OP.subtract, op1=OP.mult)
            hn1 = small.tile([P, D], BF16, tag="hn1")
            nc.gpsimd.tensor_tensor(out=hn1, in0=zh, in1=g0b, op=OP.mult)
            be0S = small.tile([P, D], BF16, tag="be0S")
            nc.gpsimd.tensor_scalar_mul(
                out=be0S, in0=be0b, scalar1=dis_sb[:, j:j + 1])
            nc.gpsimd.tensor_tensor(
                out=hnS[:, j, :], in0=hn1, in1=be0S, op=OP.add)

        # ---- G^T = dis_n * (HnS^T @ Ahat^T)  [d, n] bf16 ---------------
        GT = persist.tile([P, N], BF16, tag="GT")
        with tc.tile_pool(name="psG", bufs=2, space=bass.MemorySpace.PSUM) as psG:
            for c in range(2):
                g0t = psG.tile([P, 512], F32, tag="g0t")
                for j in range(NT):
                    nc.tensor.matmul(
                        g0t, hnS[:, j, :], aT[j][:, c * 512:(c + 1) * 512],
                        start=(j == 0), stop=(j == NT - 1))
                nc.vector.tensor_tensor(
                    out=GT[:, c * 512:(c + 1) * 512], in0=g0t,
                    in1=disrow[:, c * 512:(c + 1) * 512], op=OP.mult)

        if DEBUG_TAPS:
            nc.sync.dma_start(out=taps["t_disrow"][:, :], in_=disrow)
            nc.sync.dma_start(out=taps["t_GT"][:, :], in_=GT)
            for _i in range(NT):
                nc.sync.dma_start(out=taps["t_aT"][:, _i, :], in_=aT[_i])
            nc.sync.dma_start(out=taps["t_hnS"][:, :, :], in_=hnS)

        # ---- V = GT^T Wv + bv (vna with ones column), v = GT^T qb ------
        vna = persist.tile([P, NT, HEADS, DV], BF16, tag="vna")
        v_sb = persist.tile([P, NT, HEADS], F32, tag="v_sb")
        with tc.tile_pool(name="psV", bufs=2, space=bass.MemorySpace.PSUM) as psV, \
             tc.tile_pool(name="psv", bufs=2, space=bass.MemorySpace.PSUM) as psv:
            for i in range(NT):
                vq = psv.tile([P, HEADS], F32, tag="vq")
                nc.tensor.matmul(
                    vq, GT[:, i * P:(i + 1) * P], qb_sb, start=True, stop=True)
                nc.vector.tensor_copy(out=v_sb[:, i, :], in_=vq)
                for c in range(2):
                    vp = psV.tile([P, 512], F32, tag="vp")
                    nc.tensor.matmul(
                        vp, GT[:, i * P:(i + 1) * P],
                        wv_sb[:, c * 512:(c + 1) * 512], start=True, stop=True)
                    nc.vector.tensor_tensor(
                        out=vna[:, i, c * 4:(c + 1) * 4, 0:DV],
                        in0=vp.rearrange("p (a b) -> p a b", a=4),
                        in1=bvb[:, c * 512:(c + 1) * 512].rearrange(
                            "p (a b) -> p a b", a=4),
                        op=OP.add)

        # ---- phase D: attention, software-pipelined over heads ---------
        # PE stream per head: [P_h matmuls] -> [PV of head h-1] -> [S+B of h],
        # so the tensor engine never waits for the ACT exp chain.
        mhcT = persist.tile([P, HEADS, N], BF16, tag="mhcT")
        bt_tiles = {}
        et_tiles = {}
        psb_tiles = {}
        BFLAT = NT * N + 512  # flat per-head B^T + zero pad for DoubleRow view
        with tc.tile_pool(name="btp", bufs=3) as btp, \
             tc.tile_pool(name="pp", bufs=2) as ppool, \
             tc.tile_pool(name="etp", bufs=2) as etp, \
             tc.tile_pool(name="zrp", bufs=6) as zrp, \
             tc.tile_pool(name="psS", bufs=2, space=bass.MemorySpace.PSUM) as psS, \
             tc.tile_pool(name="psM", bufs=2, space=bass.MemorySpace.PSUM) as psM:

            def load_bt(h):
                t = btp.tile([P, BFLAT], FP8, tag="bt", name=f"bt{h}")
                nc.gpsimd.memset(t[:, NT * N:], 0.0)
                nc.sync.dma_start(
                    out=t[:, 0:NT * N].rearrange("p (t n) -> p t n", t=NT),
                    in_=bt_in[h].rearrange("(t p) n -> p t n", p=P))
                bt_tiles[h] = t

            def prep_head(h):
                pp = psS.tile([P, N], F32, tag="st", name=f"pp{h}")
                for c in range(2):
                    nc.tensor.matmul(
                        pp[:, c * 512:(c + 1) * 512], mh_sb[:, h, :],
                        GT[:, c * 512:(c + 1) * 512], start=True, stop=True)
                p_sb = ppool.tile([P, N], BF16, tag="p_sb", name=f"psb{h}")
                for c in range(2):
                    # head 0/1 copies on ACT (idle at startup; DVE is busy
                    # with the V-phase adds then)
                    if h < 2:
                        nc.scalar.copy(
                            out=p_sb[:, c * 512:(c + 1) * 512],
                            in_=pp[:, c * 512:(c + 1) * 512])
                    else:
                        nc.vector.tensor_copy(
                            out=p_sb[:, c * 512:(c + 1) * 512],
                            in_=pp[:, c * 512:(c + 1) * 512])
                psb_tiles[h] = p_sb

            def s_loop(h, j0, j1):
                if j0 == 0:
                    et_tiles[h] = etp.tile(
                        [P, NT, N], BF16, tag="et", name=f"et{h}")
                et = et_tiles[h]
                btt = bt_tiles[h]
                p_sb = psb_tiles[h]
                for j in range(j0, j1):
                    st = psS.tile([P, N], F32, tag="st", name=f"st{h}_{j}")
                    for c in range(2):
                        nc.tensor.matmul(
                            st[:, c * 512:(c + 1) * 512],
                            p_sb[:, j * P:(j + 1) * P],
                            GT[:, c * 512:(c + 1) * 512],
                            start=True, stop=False)
                    for c in range(2):
                        if USE_DOUBLEROW:
                            rv = btt[:, j * N + c * 512:
                                     j * N + c * 512 + 1024].rearrange(
                                "p (a b) -> p a b", a=2)
                            nc.tensor.matmul(
                                st[:, c * 512:(c + 1) * 512], id2a, rv,
                                start=False, stop=True,
                                perf_mode=PM.DoubleRow, skip_group_check=True)
                        else:
                            nc.tensor.matmul(
                                st[:, c * 512:(c + 1) * 512], id8,
                                btt[:, j * N + c * 512:j * N + (c + 1) * 512],
                                start=False, stop=True, skip_group_check=True)
                    nc.scalar.activation(
                        out=et[:, j, :], in_=st, func=AF.Exp,
                        bias=v_sb[:, j, h:h + 1])
                if DEBUG_TAPS and h == 0:
                    nc.sync.dma_start(out=taps["t_et0"][:, :, :], in_=et)

            def pv_half(h, half):
                et = et_tiles[h]
                if True:
                    pms = []
                    mvv = small.tile([P, 4, 2], F32, tag="mvv",
                                     name=f"mvv{h}_{half}")
                    for pi in range(2):
                        pm = psM.tile([P, 2, 512], F32, tag="pm",
                                      name=f"pm{h}_{half}_{pi}")
                        pms.append(pm)
                        for ii in range(2):
                            i = half * 4 + pi * 2 + ii
                            for j in range(NT):
                                nc.tensor.matmul(
                                    pm[:, ii, 0:DV],
                                    et[:, j, i * P:(i + 1) * P],
                                    vna[:, j, h, :],
                                    start=(j == 0), stop=(j == NT - 1))
                        for ii in range(2):
                            s6 = small.tile([P, 6], F32, tag="s6a",
                                            name=f"s6a{h}_{half}_{pi}_{ii}")
                            nc.vector.bn_stats(out=s6, in_=pm[:, ii, 0:DV])
                            nc.vector.bn_aggr(
                                out=mvv[:, pi * 2 + ii, :], in_=s6)
                    t3 = small.tile([P, 4], F32, tag="t3", name=f"t3{h}_{half}")
                    nc.vector.tensor_scalar_add(
                        out=t3, in0=mvv[:, :, 1], scalar1=EPS)
                    rstd = small.tile([P, 4], F32, tag="rstda",
                                      name=f"rsd{h}_{half}")
                    _rsqrt_pool(nc, small, rstd, t3, 4, f"a{h}_{half}", {
                        "one": one8[:, 0:4], "magic": magic8[:, 0:4],
                        "c15": c158[:, 0:4], "cm05": cm058[:, 0:4]})
                    for pi in range(2):
                        for ii in range(2):
                            i = half * 4 + pi * 2 + ii
                            zrow = zrp.tile([P, DV], BF16, tag="zrow",
                                            name=f"z{h}_{half}_{pi}_{ii}")
                            nc.vector.tensor_scalar(
                                out=zrow, in0=pms[pi][:, ii, 0:DV],
                                scalar1=mvv[:, pi * 2 + ii, 0:1],
                                scalar2=rstd[:, pi * 2 + ii:pi * 2 + ii + 1],
                                op0=OP.subtract, op1=OP.mult)
                            nc.sync.dma_start_transpose(
                                out=mhcT[:, h, i * P:(i + 1) * P], in_=zrow)

            load_bt(0)
            load_bt(1)
            prep_head(0)
            s_loop(0, 0, NT)
            for h in range(1, HEADS):
                prep_head(h)
                if h + 1 < HEADS:
                    load_bt(h + 1)
                s_loop(h, 0, NT // 2)
                pv_half(h - 1, 0)
                s_loop(h, NT // 2, NT)
                pv_half(h - 1, 1)
            pv_half(HEADS - 1, 0)
            pv_half(HEADS - 1, 1)

        # ---- phase E: output projection + MLP --------------------------
        orow_sb = persist.tile([P, NT, D], F32, tag="orow")
        mvO = small.tile([P, NT, 2], F32, tag="mvO")
        with tc.tile_pool(name="psE", bufs=2, space=bass.MemorySpace.PSUM) as psE:
            for i in range(NT):
                op = psE.tile([P, D], F32, tag="op")
                for hh in range(HEADS):
                    nc.tensor.matmul(
                        op, mhcT[:, hh, i * P:(i + 1) * P], wo_sb[:, hh, :],
                        start=(hh == 0), stop=False)
                nc.tensor.matmul(
                    op, ones1, bo_row, start=False, stop=True,
                    skip_group_check=True)
                nc.vector.tensor_tensor(
                    out=orow_sb[:, i, :], in0=op, in1=h_sb[:, i, :], op=OP.add)
                s6 = small.tile([P, 6], F32, tag="s6o")
                nc.vector.bn_stats(out=s6, in_=orow_sb[:, i, :])
                nc.vector.bn_aggr(out=mvO[:, i, :], in_=s6)
        if DEBUG_TAPS:
            nc.sync.dma_start(out=taps["t_orow"][:, :, :], in_=orow_sb)
        tvO = small.tile([P, NT], F32, tag="tvO")
        nc.vector.tensor_scalar_add(
            out=tvO, in0=mvO[:, :, 1], scalar1=EPS)
        rstdO = small.tile([P, NT], F32, tag="rstdO")
        _rsqrt(nc, nc.vector, small, rstdO, tvO, NT, "O", magic8)
        oz = persist.tile([P, NT, D], BF16, tag="oz")
        oT = persist.tile([P, N], BF16, tag="oT")
        f1_sb = persist.tile([P, NT, D], F32, tag="f1_sb")
        for i in range(NT):
            nc.vector.tensor_scalar(
                out=oz[:, i, :], in0=orow_sb[:, i, :], scalar1=mvO[:, i, 0:1],
                scalar2=rstdO[:, i:i + 1], op0=OP.subtract, op1=OP.mult)
            nc.sync.dma_start_transpose(
                out=oT[:, i * P:(i + 1) * P], in_=oz[:, i, :])
            # f1 = oz*g2 + be23 computed early on Pool, overlapping the MLP
            nc.gpsimd.tensor_tensor(
                out=f1_sb[:, i, :], in0=oz[:, i, :], in1=g2b, op=OP.mult)
            nc.gpsimd.tensor_tensor(
                out=f1_sb[:, i, :], in0=f1_sb[:, i, :], in1=be23b, op=OP.add)

        r1T = persist.tile([P, N], BF16, tag="r1T")
        r2T = persist.tile([P, N], BF16, tag="r2T")
        with tc.tile_pool(name="psE2", bufs=2, space=bass.MemorySpace.PSUM) as psE2:
            for c in range(2):
                ps = psE2.tile([P, 512], F32, tag="ps2")
                nc.tensor.matmul(
                    ps, w1_sb, oT[:, c * 512:(c + 1) * 512], start=True, stop=True)
                nc.scalar.activation(
                    out=r1T[:, c * 512:(c + 1) * 512], in_=ps, func=AF.Relu,
                    bias=b1p)
            for c in range(2):
                ps = psE2.tile([P, 512], F32, tag="ps2")
                nc.tensor.matmul(
                    ps, w2_sb, r1T[:, c * 512:(c + 1) * 512], start=True, stop=True)
                nc.scalar.activation(
                    out=r2T[:, c * 512:(c + 1) * 512], in_=ps, func=AF.Relu,
                    bias=b2p)

        r2n = persist.tile([P, NT, D], BF16, tag="r2n")
        mvR = small.tile([P, NT, 2], F32, tag="mvR")
        for i in range(NT):
            nc.sync.dma_start_transpose(
                out=r2n[:, i, :], in_=r2T[:, i * P:(i + 1) * P])
            s6 = small.tile([P, 6], F32, tag="s6r")
            nc.vector.bn_stats(out=s6, in_=r2n[:, i, :])
            nc.vector.bn_aggr(out=mvR[:, i, :], in_=s6)
        if DEBUG_TAPS:
            nc.sync.dma_start(out=taps["t_r2n"][:, :, :], in_=r2n)
        tvR = small.tile([P, NT], F32, tag="tvR")
        nc.vector.tensor_scalar_add(
            out=tvR, in0=mvR[:, :, 1], scalar1=EPS)
        rstdR = small.tile([P, NT], F32, tag="rstdR")
        _rsqrt(nc, nc.vector, small, rstdR, tvR, NT, "R", magic8)

        out_sb = persist.tile([P, NT, D], F32, tag="osb")
        for i in range(NT):
            zr = small.tile([P, D], F32, tag="zr", name=f"zr{i}")
            nc.vector.tensor_scalar(
                out=zr, in0=r2n[:, i, :], scalar1=mvR[:, i, 0:1],
                scalar2=rstdR[:, i:i + 1], op0=OP.subtract, op1=OP.mult)
            f2 = small.tile([P, D], F32, tag="f2", name=f"f2{i}")
            nc.vector.tensor_tensor(out=f2, in0=zr, in1=g3b, op=OP.mult)
            nc.vector.tensor_tensor(
                out=out_sb[:, i, :], in0=f1_sb[:, i, :], in1=f2, op=OP.add)
        nc.sync.dma_start(
            out=out_dram.rearrange("(t p) d -> p t d", p=P), in_=out_sb)

    nc.compile()
    return nc


def _get_program():
    if "nc" not in _prog_cache:
        _prog_cache["nc"] = _build_program()
    return _prog_cache["nc"]


def kernel(**inputs):
    nc = _get_program()
    f32 = np.float32
    bf16 = ml_dtypes.bfloat16
    fp8 = ml_dtypes.float8_e4m3

    H = np.asarray(inputs["H"], dtype=f32)
    A = np.asarray(inputs["A"], dtype=f32)
    WQ = np.asarray(inputs["W_Q"], dtype=f32)
    WK = np.asarray(inputs["W_K"], dtype=f32)
    WV = np.asarray(inputs["W_V"], dtype=f32)
    WO = np.asarray(inputs["W_O"], dtype=f32)
    bQ = np.asarray(inputs["b_Q"], dtype=f32)
    g1 = np.asarray(inputs["g1"], dtype=f32)
    be1 = np.asarray(inputs["be1"], dtype=f32)
    g2 = np.asarray(inputs["g2"], dtype=f32)
    be2 = np.asarray(inputs["be2"], dtype=f32)
    W1 = np.asarray(inputs["W1"], dtype=f32)
    b1 = np.asarray(inputs["b1"], dtype=f32)

    mh = np.stack([
        DK * WK[:, h * DV:(h + 1) * DV] @ WQ[:, h * DV:(h + 1) * DV].T
        for h in range(HEADS)
    ]).astype(bf16)
    qb = np.stack([
        DK * WK[:, h * DV:(h + 1) * DV] @ bQ[h * DV:(h + 1) * DV]
        for h in range(HEADS)
    ], axis=1).astype(bf16)  # [D, HEADS]
    WOp = (g1[:, None] * WO.reshape(HEADS, DV, D)).astype(bf16)  # [H, DV, D]
    bo = (be1 @ WO.reshape(HEADS, DV, D).sum(0)).reshape(1, D).astype(bf16)
    W1p = (g2[:, None] * W1).astype(bf16)
    b1p = (be2 @ W1 + b1).reshape(D, 1).astype(f32)
    be23 = (be2 + np.asarray(inputs["be3"], dtype=f32)).astype(f32)

    BT = np.ascontiguousarray(
        np.asarray(inputs["B_bias"], dtype=f32).transpose(0, 2, 1)).astype(fp8)

    wp32 = np.zeros((P, 1666), f32)
    wp32[:, 0:128] = np.asarray(inputs["g0"], dtype=f32)[None, :]
    wp32[:, 128:256] = np.asarray(inputs["be0"], dtype=f32)[None, :]
    wp32[:, 256:384] = g2[None, :]
    wp32[:, 384:512] = np.asarray(inputs["g3"], dtype=f32)[None, :]
    wp32[:, 512:640] = be23[None, :]
    wp32[:, 640:1664] = np.asarray(inputs["b_V"], dtype=f32)[None, :]
    wp32[:, 1664] = b1p[:, 0]
    wp32[:, 1665] = np.asarray(inputs["b2"], dtype=f32)
    wp16 = np.zeros((P, 3464), bf16)
    wp16[:, 0:1024] = mh.transpose(1, 0, 2).reshape(P, HEADS * D)
    wp16[:, 1024:2048] = WV.astype(bf16)
    wp16[:, 2048:3072] = WOp.transpose(1, 0, 2).reshape(P, HEADS * D)
    wp16[:, 3072:3200] = W1p
    wp16[:, 3200:3328] = np.asarray(inputs["W2"], dtype=f32).astype(bf16)
    wp16[:, 3328:3336] = qb
    wp16[:, 3336:3464] = bo[None, 0, :]
    base = {
        "bt": BT,
        "wp32": wp32,
        "wp16": np.ascontiguousarray(wp16),
    }

    in_maps = []
    for c in range(B):
        m = dict(base)
        m["h"] = np.ascontiguousarray(H[c])
        m["a16"] = np.ascontiguousarray(A[c]).astype(bf16)
        in_maps.append(m)

    res = run_bass_kernel_spmd(nc, in_maps, list(range(B)))
    if DEBUG_TAPS:
        _prog_cache["taps"] = res.results
    out = np.stack([res.results[c]["out"] for c in range(B)], axis=0)
    return out.astype(np.float32)


if __name__ == "__main__":
    _get_program()
    print("program built ok")


# revision 38
# speedup vs baseline: 1.0610x; 1.0610x over previous
"""Trainium2 Bass kernel for nn_GRIC_31550829756424 (GCN-attention block).

Data-parallel over batch: 8 batches -> 8 NeuronCores, one full batch per core.

Algebraic restructure vs the straightforward lowering:
  Q = adj @ (Hn Wq) = (adj Hn) Wq  ==>  G = adj_norm @ Hn computed ONCE.
  S^T_h = G M2_h G^T with M2_h = dk*Wk_h Wq_h^T folded on host, so scores need
  no Q/K materialization. Row-constant score-bias terms cancel in softmax and
  are dropped; the column term v = G @ (dk Wk_h b_Q,h) rides the exp bias.
  g1/be1 fold into W_O / a bias row; g2/be2 fold into W1/b1 for the MLP.
  B_bias is pre-transposed + fp8-cast on host and added into PSUM by the PE
  via identity matmuls (DoubleRow fp8: 0.5 cyc/row).
  LayerNorm rstd uses exp(-0.5*ln(x)) so the ACT engine never leaves the
  {exp, ln, relu} activation table (no table reloads).
  All transposes ride the DMA XBAR (A^T from DRAM, z->mhcT, oz->oT, r2T back).

Self-contained: hardcodes all shapes; imports only the in-container concourse
stack.
"""

import sys

sys.path.insert(0, "/opt/trn_rl_repo")

import numpy as np
import ml_dtypes
from contextlib import ExitStack

import concourse.bass as bass
import concourse.tile as tile
from concourse import bacc
from concourse import mybir
from concourse.bass_utils import run_bass_kernel_spmd

F32 = mybir.dt.float32
BF16 = mybir.dt.bfloat16
FP8 = mybir.dt.float8e4
AF = mybir.ActivationFunctionType
OP = mybir.AluOpType
AX = mybir.AxisListType
PM = mybir.MatmulPerfMode

B = 8
N = 1024
D = 128
HEADS = 8
DV = 128
HD = HEADS * DV  # 1024
P = 128
NT = N // P  # 8 tiles of 128 rows
DK = 1.0 / float(np.sqrt(np.float32(D)))
EPS = 1e-5

USE_DOUBLEROW = True
DEBUG_TAPS = False

_prog_cache = {}


def _bcast_load(nc, dst, src):
    """DMA-load 1D DRAM vector src [W] replicated across all P partitions of
    dst [P, W] (issued on the ACT HWDGE queue)."""
    rep = bass.AP(tensor=src.tensor, offset=src.offset, ap=[[0, P]] + list(src.ap))
    nc.scalar.dma_start(out=dst, in_=rep)


U32 = mybir.dt.uint32
_RSQRT_MAGIC = 0x5F3759DF


def _rsqrt_pool(nc, pool, out_ap, in_ap, w, tag, cns):
    """Pool-engine rsqrt via tensor_tensor-only ops (Pool rejects
    TensorScalarPtr). cns = dict of const tiles sliced to [P, w]:
    one (u32 1), magic, c15 (1.5), cm05 (-0.5)."""
    eng = nc.gpsimd
    y = pool.tile([P, w], F32, name=f"rp_y{tag}", tag=f"rp_y{w}")
    ib = pool.tile([P, w], U32, name=f"rp_i{tag}", tag=f"rp_i{w}")
    # seed on DVE (Pool shifts require u64 outputs); iterations on Pool
    nc.vector.tensor_scalar(
        out=ib, in0=in_ap.bitcast(U32), scalar1=1, scalar2=None,
        op0=OP.logical_shift_right)
    nc.vector.tensor_tensor(out=y.bitcast(U32), in0=cns["magic"], in1=ib,
                            op=OP.subtract)
    for it in range(2):
        a = pool.tile([P, w], F32, name=f"rp_a{tag}_{it}", tag=f"rp_a{w}")
        eng.tensor_tensor(out=a, in0=y, in1=y, op=OP.mult)
        eng.tensor_tensor(out=a, in0=a, in1=in_ap, op=OP.mult)
        eng.tensor_tensor(out=a, in0=a, in1=cns["cm05"], op=OP.mult)
        eng.tensor_tensor(out=a, in0=a, in1=cns["c15"], op=OP.add)
        dst = out_ap if it == 1 else y
        eng.tensor_tensor(out=dst, in0=y, in1=a, op=OP.mult)


def _rsqrt(nc, eng, pool, out_ap, in_ap, w, tag, magic):
    """out = 1/sqrt(in) elementwise on [P, w] f32 via bit-hack seed + 2 Newton
    iterations. Runs entirely on `eng` (vector or gpsimd) — avoids the ACT
    table thrash that Sqrt/Ln would cause next to Exp."""
    y = pool.tile([P, w], F32, name=f"rq_y{tag}", tag=f"rq_y{w}")
    ib = pool.tile([P, w], U32, name=f"rq_i{tag}", tag=f"rq_i{w}")
    eng.tensor_scalar(
        out=ib, in0=in_ap.bitcast(U32), scalar1=1, scalar2=None,
        op0=OP.logical_shift_right)
    eng.tensor_tensor(out=y.bitcast(U32), in0=magic, in1=ib, op=OP.subtract)
    for it in range(2):
        a = pool.tile([P, w], F32, name=f"rq_a{tag}_{it}", tag=f"rq_a{w}")
        eng.tensor_tensor(out=a, in0=y, in1=y, op=OP.mult)
        eng.tensor_tensor(out=a, in0=a, in1=in_ap, op=OP.mult)
        eng.tensor_scalar(
            out=a, in0=a, scalar1=-0.5, scalar2=1.5, op0=OP.mult, op1=OP.add)
        dst = out_ap if it == 1 else y
        eng.tensor_tensor(out=dst, in0=y, in1=a, op=OP.mult)


def _build_program():
    nc = bacc.Bacc(None)

    h_in = nc.declare_dram_parameter("h", [N, D], F32, isOutput=False)
    a16_in = nc.declare_dram_parameter("a16", [N, N], BF16, isOutput=False)
    bt_in = nc.declare_dram_parameter("bt", [HEADS, N, N], FP8, isOutput=False)
    wp32_in = nc.declare_dram_parameter("wp32", [P, 1666], F32, isOutput=False)
    wp16_in = nc.declare_dram_parameter("wp16", [P, 3464], BF16, isOutput=False)
    out_dram = nc.declare_dram_parameter("out", [N, D], F32, isOutput=True)
    taps = {}
    if DEBUG_TAPS:
        taps["t_disrow"] = nc.declare_dram_parameter(
            "t_disrow", [P, N], F32, isOutput=True)
        taps["t_GT"] = nc.declare_dram_parameter(
            "t_GT", [P, N], BF16, isOutput=True)
        taps["t_aT"] = nc.declare_dram_parameter(
            "t_aT", [P, NT, N], BF16, isOutput=True)
        taps["t_hnS"] = nc.declare_dram_parameter(
            "t_hnS", [P, NT, D], BF16, isOutput=True)
        taps["t_vna"] = nc.declare_dram_parameter(
            "t_vna", [P, NT, HEADS, DV + 1], BF16, isOutput=True)
        taps["t_et0"] = nc.declare_dram_parameter(
            "t_et0", [P, NT, N], BF16, isOutput=True)
        taps["t_mhcT"] = nc.declare_dram_parameter(
            "t_mhcT", [P, HEADS, N], BF16, isOutput=True)
        taps["t_orow"] = nc.declare_dram_parameter(
            "t_orow", [P, NT, D], F32, isOutput=True)
        taps["t_r2n"] = nc.declare_dram_parameter(
            "t_r2n", [P, NT, D], BF16, isOutput=True)
    disr = nc.dram_tensor("disr", [N], F32, kind="Internal")

    with tile.TileContext(nc) as tc, ExitStack() as ctx:
        consts = ctx.enter_context(tc.tile_pool(name="consts", bufs=1))
        persist = ctx.enter_context(tc.tile_pool(name="persist", bufs=1))
        small = ctx.enter_context(tc.tile_pool(name="small", bufs=12))

        # ---- constants -------------------------------------------------
        # diag-fix masks in bf16: omi = 1 - I, identb = I
        omi = consts.tile([P, P], BF16)
        nc.gpsimd.memset(omi, 1.0)
        nc.gpsimd.affine_select(
            out=omi, in_=omi, compare_op=OP.not_equal, fill=0.0,
            base=0, pattern=[[-1, P]], channel_multiplier=1)
        identb = consts.tile([P, P], BF16)
        nc.gpsimd.memset(identb, 0.0)
        nc.gpsimd.affine_select(
            out=identb, in_=identb, compare_op=OP.not_equal, fill=1.0,
            base=0, pattern=[[-1, P]], channel_multiplier=1)
        if USE_DOUBLEROW:
            # [I | 0] and [0 | I] fp8 stationaries for DoubleRow bias-adds
            id2a = consts.tile([P, 2, P], FP8)
            nc.gpsimd.memset(id2a, 0.0)
            nc.vector.tensor_copy(out=id2a[:, 0, :], in_=identb)
            id2b = consts.tile([P, 2, P], FP8)
            nc.gpsimd.memset(id2b, 0.0)
            nc.vector.tensor_copy(out=id2b[:, 1, :], in_=identb)
        else:
            id8 = consts.tile([P, P], FP8)
            nc.vector.tensor_copy(out=id8, in_=identb)
        ones1 = consts.tile([1, P], BF16)
        nc.vector.memset(ones1, 1.0)
        magic8 = consts.tile([P, NT], U32)
        nc.vector.memset(magic8, _RSQRT_MAGIC)
        one8 = consts.tile([P, NT], U32)
        nc.vector.memset(one8, 1)
        c158 = consts.tile([P, NT], F32)
        nc.vector.memset(c158, 1.5)
        cm058 = consts.tile([P, NT], F32)
        nc.vector.memset(cm058, -0.5)
        onescol = consts.tile([P, 1], BF16)
        nc.vector.memset(onescol, 1.0)

        # ---- phase A/B: H + A + packed-weight loads ---------------------
        h_sb = persist.tile([P, NT, D], F32, tag="h")
        nc.scalar.dma_start(out=h_sb, in_=h_in.rearrange("(t p) d -> p t d", p=P))
        wp32 = consts.tile([P, 1666], F32)
        nc.scalar.dma_start(out=wp32, in_=wp32_in[:, :])
        wp16 = consts.tile([P, 3464], BF16)
        nc.scalar.dma_start(out=wp16, in_=wp16_in[:, :])
        g0b = wp32[:, 0:128]
        be0b = wp32[:, 128:256]
        g2b = wp32[:, 256:384]
        g3b = wp32[:, 384:512]
        be23b = wp32[:, 512:640]
        bvb = wp32[:, 640:1664]
        b1p = wp32[:, 1664:1665]
        b2p = wp32[:, 1665:1666]
        mh_sb = wp16[:, 0:1024].rearrange("p (hh d) -> p hh d", hh=HEADS)
        wv_sb = wp16[:, 1024:2048]
        wo_sb = wp16[:, 2048:3072].rearrange("p (hh d) -> p hh d", hh=HEADS)
        w1_sb = wp16[:, 3072:3200]
        w2_sb = wp16[:, 3200:3328]
        qb_sb = wp16[:, 3328:3336]
        bo_row = wp16[0:1, 3336:3464]

        # A loads: natural chunks (scoped) feed DVE rowsums; A^T via XBAR
        # into 8 SEPARATE tiles (a shared tile serializes the XBARs against
        # the diag-fix RMWs through tile-level dep tracking).
        rs_all = small.tile([P, NT], F32, tag="rs_all")
        mvH = small.tile([P, NT, 2], F32, tag="mvH")
        anat_ctx = tc.tile_pool(name="anatp", bufs=1)
        anatp = anat_ctx.__enter__()
        anat = anatp.tile([P, NT, N], BF16, tag="anat")
        aTt = persist.tile([P, NT, N], BF16, tag="aTt")  # [m-chunk, n]
        aT = [aTt[:, i, :] for i in range(NT)]
        psT_ctx = tc.tile_pool(name="psT", bufs=4, space=bass.MemorySpace.PSUM)
        psT = psT_ctx.__enter__()
        for j in range(NT):
            an = anat[:, j, :]
            nc.sync.dma_start(out=an, in_=a16_in[j * P:(j + 1) * P, :])
            db = anat[:, j, j * P:(j + 1) * P]
            nc.gpsimd.tensor_tensor(out=db, in0=db, in1=omi, op=OP.mult)
            nc.gpsimd.tensor_tensor(out=db, in0=db, in1=identb, op=OP.add)
            nc.vector.reduce_sum(
                out=rs_all[:, j:j + 1], in_=an, axis=AX.X)
            # transpose the 8 blocks of this natural chunk on the (idle) PE;
            # one strided 512-wide copy per 4 blocks, alternating ACT/DVE
            for g in range(2):
                pt = psT.tile([P, 4, P], BF16, tag="pt", name=f"pt{j}_{g}")
                for ii in range(4):
                    i = g * 4 + ii
                    nc.tensor.transpose(
                        pt[:, ii, :], anat[:, j, i * P:(i + 1) * P], identb)
                dst = aTt[:, g * 4:(g + 1) * 4, j * P:(j + 1) * P]
                if (j + g) % 2 == 0:
                    nc.scalar.copy(out=dst, in_=pt)
                else:
                    nc.vector.tensor_copy(out=dst, in_=pt)
        psT_ctx.__exit__(None, None, None)
        for j in range(NT):
            # H LN stats (independent of A)
            s6 = small.tile([P, 6], F32, tag="s6h", name=f"s6h{j}")
            nc.vector.bn_stats(out=s6, in_=h_sb[:, j, :])
            nc.vector.bn_aggr(out=mvH[:, j, :], in_=s6)

        # dis = rsqrt(max(rowsum, 1))  [P, NT]
        dmax = small.tile([P, NT], F32, tag="dmax")
        nc.vector.tensor_scalar_max(out=dmax, in0=rs_all, scalar1=1.0)
        dis_sb = small.tile([P, NT], F32, tag="dis")
        _rsqrt(nc, nc.vector, small, dis_sb, dmax, NT, "dis", magic8)
        anat_ctx.__exit__(None, None, None)

        # HnS = dis * LN(H):  rstd2 = rstdH*dis
        tvH = small.tile([P, NT], F32, tag="tvH")
        nc.vector.tensor_scalar_add(
            out=tvH, in0=mvH[:, :, 1], scalar1=EPS)
        rstdH = small.tile([P, NT], F32, tag="rstdH")
        _rsqrt(nc, nc.vector, small, rstdH, tvH, NT, "H", magic8)
        rstd2 = small.tile([P, NT], F32, tag="rstd2")
        nc.vector.tensor_tensor(out=rstd2, in0=rstdH, in1=dis_sb, op=OP.mult)

        # disrow [P, N] broadcast of dis in n-order via DRAM round-trip
        nc.sync.dma_start(out=disr.rearrange("(t p) -> p t", p=P), in_=dis_sb)
        disrow = persist.tile([P, N], F32, tag="disrow")
        rep = bass.AP(
            tensor=disr[:].tensor, offset=disr[:].offset,
            ap=[[0, P]] + list(disr[:].ap))
        nc.sync.dma_start(out=disrow, in_=rep)

        hnS = persist.tile([P, NT, D], BF16, tag="hnS")
        for j in range(NT):
            zh = small.tile([P, D], F32, tag="zh")
            nc.vector.tensor_scalar(
                out=zh, in0=h_sb[:, j, :], scalar1=mvH[:, j, 0:1],
                scalar2=rstd2[:, j:j + 1], op0=OP.subtract, op1=OP.mult)
            hn1 = small.tile([P, D], BF16, tag="hn1")
            nc.gpsimd.tensor_tensor(out=hn1, in0=zh, in1=g0b, op=OP.mult)
            be0S = small.tile([P, D], BF16, tag="be0S")
            nc.gpsimd.tensor_scalar_mul(
                out=be0S, in0=be0b, scalar1=dis_sb[:, j:j + 1])
            nc.gpsimd.tensor_tensor(
                out=hnS[:, j, :], in0=hn1, in1=be0S, op=OP.add)

        # ---- G^T = dis_n * (HnS^T @ Ahat^T)  [d, n] bf16 ---------------
        GT = persist.tile([P, N], BF16, tag="GT")
        with tc.tile_pool(name="psG", bufs=2, space=bass.MemorySpace.PSUM) as psG:
            for c in range(2):
                g0t = psG.tile([P, 512], F32, tag="g0t")
                for j in range(NT):
                    nc.tensor.matmul(
                        g0t, hnS[:, j, :], aT[j][:, c * 512:(c + 1) * 512],
                        start=(j == 0), stop=(j == NT - 1))
                nc.vector.tensor_tensor(
                    out=GT[:, c * 512:(c + 1) * 512], in0=g0t,
                    in1=disrow[:, c * 512:(c + 1) * 512], op=OP.mult)

        if DEBUG_TAPS:
            nc.sync.dma_start(out=taps["t_disrow"][:, :], in_=disrow)
            nc.sync.dma_start(out=taps["t_GT"][:, :], in_=GT)
            for _i in range(NT):
                nc.sync.dma_start(out=taps["t_aT"][:, _i, :], in_=aT[_i])
            nc.sync.dma_start(out=taps["t_hnS"][:, :, :], in_=hnS)

        # ---- V = GT^T Wv + bv (vna with ones column), v = GT^T qb ------
        vna = persist.tile([P, NT, HEADS, DV], BF16, tag="vna")
        v_sb = persist.tile([P, NT, HEADS], F32, tag="v_sb")
        with tc.tile_pool(name="psV", bufs=2, space=bass.MemorySpace.PSUM) as psV, \
             tc.tile_pool(name="psv", bufs=2, space=bass.MemorySpace.PSUM) as psv:
            for i in range(NT):
                vq = psv.tile([P, HEADS], F32, tag="vq")
                nc.tensor.matmul(
                    vq, GT[:, i * P:(i + 1) * P], qb_sb, start=True, stop=True)
                nc.vector.tensor_copy(out=v_sb[:, i, :], in_=vq)
                for c in range(2):
                    vp = psV.tile([P, 512], F32, tag="vp")
                    nc.tensor.matmul(
                        vp, GT[:, i * P:(i + 1) * P],
                        wv_sb[:, c * 512:(c + 1) * 512], start=True, stop=True)
                    nc.vector.tensor_tensor(
                        out=vna[:, i, c * 4:(c + 1) * 4, 0:DV],
                        in0=vp.rearrange("p (a b) -> p a b", a=4),
                        in1=bvb[:, c * 512:(c + 1) * 512].rearrange(
                            "p (a b) -> p a b", a=4),
                        op=OP.add)

        # ---- phase D: attention, software-pipelined over heads ---------
        # PE stream per head: [P_h matmuls] -> [PV of head h-1] -> [S+B of h],
        # so the tensor engine never waits for the ACT exp chain.
        mhcT = persist.tile([P, HEADS, N], BF16, tag="mhcT")
        bt_tiles = {}
        et_tiles = {}
        psb_tiles = {}
        BFLAT = NT * N + 512  # flat per-head B^T + zero pad for DoubleRow view
        with tc.tile_pool(name="btp", bufs=3) as btp, \
             tc.tile_pool(name="pp", bufs=2) as ppool, \
             tc.tile_pool(name="etp", bufs=2) as etp, \
             tc.tile_pool(name="zrp", bufs=6) as zrp, \
             tc.tile_pool(name="psS", bufs=2, space=bass.MemorySpace.PSUM) as psS, \
             tc.tile_pool(name="psM", bufs=2, space=bass.MemorySpace.PSUM) as psM:

            def load_bt(h):
                t = btp.tile([P, BFLAT], FP8, tag="bt", name=f"bt{h}")
                nc.gpsimd.memset(t[:, NT * N:], 0.0)
                nc.sync.dma_start(
                    out=t[:, 0:NT * N].rearrange("p (t n) -> p t n", t=NT),
                    in_=bt_in[h].rearrange("(t p) n -> p t n", p=P))
                bt_tiles[h] = t

            def prep_head(h):
                pp = psS.tile([P, N], F32, tag="st", name=f"pp{h}")
                for c in range(2):
                    nc.tensor.matmul(
                        pp[:, c * 512:(c + 1) * 512], mh_sb[:, h, :],
                        GT[:, c * 512:(c + 1) * 512], start=True, stop=True)
                p_sb = ppool.tile([P, N], BF16, tag="p_sb", name=f"psb{h}")
                for c in range(2):
                    # head 0/1 copies on ACT (idle at startup; DVE is busy
                    # with the V-phase adds then)
                    if h < 2:
                        nc.scalar.copy(
                            out=p_sb[:, c * 512:(c + 1) * 512],
                            in_=pp[:, c * 512:(c + 1) * 512])
                    else:
                        nc.vector.tensor_copy(
                            out=p_sb[:, c * 512:(c + 1) * 512],
                            in_=pp[:, c * 512:(c + 1) * 512])
                psb_tiles[h] = p_sb

            def s_loop(h, j0, j1):
                if j0 == 0:
                    et_tiles[h] = etp.tile(
                        [P, NT, N], BF16, tag="et", name=f"et{h}")
                et = et_tiles[h]
                btt = bt_tiles[h]
                p_sb = psb_tiles[h]
                for j in range(j0, j1):
                    st = psS.tile([P, N], F32, tag="st", name=f"st{h}_{j}")
                    for c in range(2):
                        nc.tensor.matmul(
                            st[:, c * 512:(c + 1) * 512],
                            p_sb[:, j * P:(j + 1) * P],
                            GT[:, c * 512:(c + 1) * 512],
                            start=True, stop=False)
                    for c in range(2):
                        if USE_DOUBLEROW:
                            rv = btt[:, j * N + c * 512:
                                     j * N + c * 512 + 1024].rearrange(
                                "p (a b) -> p a b", a=2)
                            nc.tensor.matmul(
                                st[:, c * 512:(c + 1) * 512], id2a, rv,
                                start=False, stop=True,
                                perf_mode=PM.DoubleRow, skip_group_check=True)
                        else:
                            nc.tensor.matmul(
                                st[:, c * 512:(c + 1) * 512], id8,
                                btt[:, j * N + c * 512:j * N + (c + 1) * 512],
                                start=False, stop=True, skip_group_check=True)
                    nc.scalar.activation(
                        out=et[:, j, :], in_=st, func=AF.Exp,
                        bias=v_sb[:, j, h:h + 1])
                if DEBUG_TAPS and h == 0:
                    nc.sync.dma_start(out=taps["t_et0"][:, :, :], in_=et)

            def pv_half(h, half):
                et = et_tiles[h]
                if True:
                    pms = []
                    mvv = small.tile([P, 4, 2], F32, tag="mvv",
                                     name=f"mvv{h}_{half}")
                    for pi in range(2):
                        pm = psM.tile([P, 2, 512], F32, tag="pm",
                                      name=f"pm{h}_{half}_{pi}")
                        pms.append(pm)
                        for ii in range(2):
                            i = half * 4 + pi * 2 + ii
                            for j in range(NT):
                                nc.tensor.matmul(
                                    pm[:, ii, 0:DV],
                                    et[:, j, i * P:(i + 1) * P],
                                    vna[:, j, h, :],
                                    start=(j == 0), stop=(j == NT - 1))
                        for ii in range(2):
                            s6 = small.tile([P, 6], F32, tag="s6a",
                                            name=f"s6a{h}_{half}_{pi}_{ii}")
                            nc.vector.bn_stats(out=s6, in_=pm[:, ii, 0:DV])
                            nc.vector.bn_aggr(
                                out=mvv[:, pi * 2 + ii, :], in_=s6)
                    t3 = small.tile([P, 4], F32, tag="t3", name=f"t3{h}_{half}")
                    nc.vector.tensor_scalar_add(
                        out=t3, in0=mvv[:, :, 1], scalar1=EPS)
                    rstd = small.tile([P, 4], F32, tag="rstda",
                                      name=f"rsd{h}_{half}")
                    _rsqrt_pool(nc, small, rstd, t3, 4, f"a{h}_{half}", {
                        "one": one8[:, 0:4], "magic": magic8[:, 0:4],
                        "c15": c158[:, 0:4], "cm05": cm058[:, 0:4]})
                    for pi in range(2):
                        for ii in range(2):
                            i = half * 4 + pi * 2 + ii
                            zrow = zrp.tile([P, DV], BF16, tag="zrow",
                                            name=f"z{h}_{half}_{pi}_{ii}")
                            nc.vector.tensor_scalar(
                                out=zrow, in0=pms[pi][:, ii, 0:DV],
                                scalar1=mvv[:, pi * 2 + ii, 0:1],
                                scalar2=rstd[:, pi * 2 + ii:pi * 2 + ii + 1],
                                op0=OP.subtract, op1=OP.mult)
                            nc.sync.dma_start_transpose(
                                out=mhcT[:, h, i * P:(i + 1) * P], in_=zrow)

            load_bt(0)
            load_bt(1)
            prep_head(0)
            prep_head(1)
            s_loop(0, 0, NT)
            for h in range(1, HEADS):
                if h + 1 < HEADS:
                    load_bt(h + 1)
                s_loop(h, 0, NT // 2)
                pv_half(h - 1, 0)
                s_loop(h, NT // 2, NT)
                if h + 1 < HEADS:
                    prep_head(h + 1)
                pv_half(h - 1, 1)
            pv_half(HEADS - 1, 0)
            pv_half(HEADS - 1, 1)

        # ---- phase E: output projection + MLP --------------------------
        orow_sb = persist.tile([P, NT, D], F32, tag="orow")
        mvO = small.tile([P, NT, 2], F32, tag="mvO")
        with tc.tile_pool(name="psE", bufs=2, space=bass.MemorySpace.PSUM) as psE:
            for i in range(NT):
                op = psE.tile([P, D], F32, tag="op")
                for hh in range(HEADS):
                    nc.tensor.matmul(
                        op, mhcT[:, hh, i * P:(i + 1) * P], wo_sb[:, hh, :],
                        start=(hh == 0), stop=False)
                nc.tensor.matmul(
                    op, ones1, bo_row, start=False, stop=True,
                    skip_group_check=True)
                nc.vector.tensor_tensor(
                    out=orow_sb[:, i, :], in0=op, in1=h_sb[:, i, :], op=OP.add)
                s6 = small.tile([P, 6], F32, tag="s6o")
                nc.vector.bn_stats(out=s6, in_=orow_sb[:, i, :])
                nc.vector.bn_aggr(out=mvO[:, i, :], in_=s6)
        if DEBUG_TAPS:
            nc.sync.dma_start(out=taps["t_orow"][:, :, :], in_=orow_sb)
        tvO = small.tile([P, NT], F32, tag="tvO")
        nc.vector.tensor_scalar_add(
            out=tvO, in0=mvO[:, :, 1], scalar1=EPS)
        rstdO = small.tile([P, NT], F32, tag="rstdO")
        _rsqrt(nc, nc.vector, small, rstdO, tvO, NT, "O", magic8)
        oz = persist.tile([P, NT, D], BF16, tag="oz")
        oT = persist.tile([P, N], BF16, tag="oT")
        f1_sb = persist.tile([P, NT, D], F32, tag="f1_sb")
        psE1_ctx = tc.tile_pool(name="psE1", bufs=2, space=bass.MemorySpace.PSUM)
        psE1 = psE1_ctx.__enter__()
        for g in range(2):
            pt = psE1.tile([P, 4, P], BF16, tag="eT", name=f"ozT{g}")
            for ii in range(4):
                i = g * 4 + ii
                nc.vector.tensor_scalar(
                    out=oz[:, i, :], in0=orow_sb[:, i, :],
                    scalar1=mvO[:, i, 0:1],
                    scalar2=rstdO[:, i:i + 1], op0=OP.subtract, op1=OP.mult)
                nc.tensor.transpose(pt[:, ii, :], oz[:, i, :], identb)
                # f1 = oz*g2 + be23 computed early on Pool, overlapping the MLP
                nc.gpsimd.tensor_tensor(
                    out=f1_sb[:, i, :], in0=oz[:, i, :], in1=g2b, op=OP.mult)
                nc.gpsimd.tensor_tensor(
                    out=f1_sb[:, i, :], in0=f1_sb[:, i, :], in1=be23b,
                    op=OP.add)
            nc.scalar.copy(
                out=oT[:, g * 512:(g + 1) * 512],
                in_=pt.rearrange("p a b -> p (a b)"))

        r1T = persist.tile([P, N], BF16, tag="r1T")
        r2T = persist.tile([P, N], BF16, tag="r2T")
        with tc.tile_pool(name="psE2", bufs=2, space=bass.MemorySpace.PSUM) as psE2:
            for c in range(2):
                ps = psE2.tile([P, 512], F32, tag="ps2")
                nc.tensor.matmul(
                    ps, w1_sb, oT[:, c * 512:(c + 1) * 512], start=True, stop=True)
                nc.scalar.activation(
                    out=r1T[:, c * 512:(c + 1) * 512], in_=ps, func=AF.Relu,
                    bias=b1p)
            for c in range(2):
                ps = psE2.tile([P, 512], F32, tag="ps2")
                nc.tensor.matmul(
                    ps, w2_sb, r1T[:, c * 512:(c + 1) * 512], start=True, stop=True)
                nc.scalar.activation(
                    out=r2T[:, c * 512:(c + 1) * 512], in_=ps, func=AF.Relu,
                    bias=b2p)

        mvR = small.tile([P, NT, 2], F32, tag="mvR")
        r2p = []
        for g in range(2):
            pt = psE1.tile([P, 4, P], BF16, tag="eT", name=f"r2T{g}")
            r2p.append(pt)
            for ii in range(4):
                i = g * 4 + ii
                nc.tensor.transpose(
                    pt[:, ii, :], r2T[:, i * P:(i + 1) * P], identb)
                s6 = small.tile([P, 6], F32, tag="s6r", name=f"s6r{i}")
                nc.vector.bn_stats(out=s6, in_=pt[:, ii, :])
                nc.vector.bn_aggr(out=mvR[:, i, :], in_=s6)
        if DEBUG_TAPS:
            for _g in range(2):
                nc.sync.dma_start(
                    out=taps["t_r2n"][:, _g * 4:(_g + 1) * 4, :], in_=r2p[_g])
        tvR = small.tile([P, NT], F32, tag="tvR")
        nc.vector.tensor_scalar_add(
            out=tvR, in0=mvR[:, :, 1], scalar1=EPS)
        rstdR = small.tile([P, NT], F32, tag="rstdR")
        _rsqrt(nc, nc.vector, small, rstdR, tvR, NT, "R", magic8)

        out_sb = persist.tile([P, NT, D], F32, tag="osb")
        for i in range(NT):
            zr = small.tile([P, D], F32, tag="zr", name=f"zr{i}")
            nc.vector.tensor_scalar(
                out=zr, in0=r2p[i // 4][:, i % 4, :], scalar1=mvR[:, i, 0:1],
                scalar2=rstdR[:, i:i + 1], op0=OP.subtract, op1=OP.mult)
            f2 = small.tile([P, D], F32, tag="f2", name=f"f2{i}")
            nc.vector.tensor_tensor(out=f2, in0=zr, in1=g3b, op=OP.mult)
            nc.vector.tensor_tensor(
                out=out_sb[:, i, :], in0=f1_sb[:, i, :], in1=f2, op=OP.add)
        psE1_ctx.__exit__(None, None, None)
        for g in range(2):
            nc.sync.dma_start(
                out=out_dram.rearrange(
                    "(t p) d -> p t d", p=P)[:, g * 4:(g + 1) * 4, :],
                in_=out_sb[:, g * 4:(g + 1) * 4, :])

    nc.compile()
    return nc


def _get_program():
    if "nc" not in _prog_cache:
        _prog_cache["nc"] = _build_program()
    return _prog_cache["nc"]


def kernel(**inputs):
    nc = _get_program()
    f32 = np.float32
    bf16 = ml_dtypes.bfloat16
    fp8 = ml_dtypes.float8_e4m3

    H = np.asarray(inputs["H"], dtype=f32)
    A = np.asarray(inputs["A"], dtype=f32)
    WQ = np.asarray(inputs["W_Q"], dtype=f32)
    WK = np.asarray(inputs["W_K"], dtype=f32)
    WV = np.asarray(inputs["W_V"], dtype=f32)
    WO = np.asarray(inputs["W_O"], dtype=f32)
    bQ = np.asarray(inputs["b_Q"], dtype=f32)
    g1 = np.asarray(inputs["g1"], dtype=f32)
    be1 = np.asarray(inputs["be1"], dtype=f32)
    g2 = np.asarray(inputs["g2"], dtype=f32)
    be2 = np.asarray(inputs["be2"], dtype=f32)
    W1 = np.asarray(inputs["W1"], dtype=f32)
    b1 = np.asarray(inputs["b1"], dtype=f32)

    mh = np.stack([
        DK * WK[:, h * DV:(h + 1) * DV] @ WQ[:, h * DV:(h + 1) * DV].T
        for h in range(HEADS)
    ]).astype(bf16)
    qb = np.stack([
        DK * WK[:, h * DV:(h + 1) * DV] @ bQ[h * DV:(h + 1) * DV]
        for h in range(HEADS)
    ], axis=1).astype(bf16)  # [D, HEADS]
    WOp = (g1[:, None] * WO.reshape(HEADS, DV, D)).astype(bf16)  # [H, DV, D]
    bo = (be1 @ WO.reshape(HEADS, DV, D).sum(0)).reshape(1, D).astype(bf16)
    W1p = (g2[:, None] * W1).astype(bf16)
    b1p = (be2 @ W1 + b1).reshape(D, 1).astype(f32)
    be23 = (be2 + np.asarray(inputs["be3"], dtype=f32)).astype(f32)

    BT = np.ascontiguousarray(
        np.asarray(inputs["B_bias"], dtype=f32).transpose(0, 2, 1)).astype(fp8)

    wp32 = np.zeros((P, 1666), f32)
    wp32[:, 0:128] = np.asarray(inputs["g0"], dtype=f32)[None, :]
    wp32[:, 128:256] = np.asarray(inputs["be0"], dtype=f32)[None, :]
    wp32[:, 256:384] = g2[None, :]
    wp32[:, 384:512] = np.asarray(inputs["g3"], dtype=f32)[None, :]
    wp32[:, 512:640] = be23[None, :]
    wp32[:, 640:1664] = np.asarray(inputs["b_V"], dtype=f32)[None, :]
    wp32[:, 1664] = b1p[:, 0]
    wp32[:, 1665] = np.asarray(inputs["b2"], dtype=f32)
    wp16 = np.zeros((P, 3464), bf16)
    wp16[:, 0:1024] = mh.transpose(1, 0, 2).reshape(P, HEADS * D)
    wp16[:, 1024:2048] = WV.astype(bf16)
    wp16[:, 2048:3072] = WOp.transpose(1, 0, 2).reshape(P, HEADS * D)
    wp16[:, 3072:3200] = W1p
    wp16[:, 3200:3328] = np.asarray(inputs["W2"], dtype=f32).astype(bf16)
    wp16[:, 3328:3336] = qb
    wp16[:, 3336:3464] = bo[None, 0, :]
    base = {
        "bt": BT,
        "wp32": wp32,
        "wp16": np.ascontiguousarray(wp16),
    }

    in_maps = []
    for c in range(B):
        m = dict(base)
        m["h"] = np.ascontiguousarray(H[c])
        m["a16"] = np.ascontiguousarray(A[c]).astype(bf16)
        in_maps.append(m)

    res = run_bass_kernel_spmd(nc, in_maps, list(range(B)))
    if DEBUG_TAPS:
        _prog_cache["taps"] = res.results
    out = np.stack([res.results[c]["out"] for c in range(B)], axis=0)
    return out.astype(np.float32)


if __name__ == "__main__":
    _get_program()
    print("program built ok")


# revision 39
# speedup vs baseline: 1.1659x; 1.0988x over previous
"""Trainium2 Bass kernel for nn_GRIC_31550829756424 (GCN-attention block).

Data-parallel over batch: 8 batches -> 8 NeuronCores, one full batch per core.

Algebraic restructure vs the straightforward lowering:
  Q = adj @ (Hn Wq) = (adj Hn) Wq  ==>  G = adj_norm @ Hn computed ONCE.
  S^T_h = G M2_h G^T with M2_h = dk*Wk_h Wq_h^T folded on host, so scores need
  no Q/K materialization. Row-constant score-bias terms cancel in softmax and
  are dropped; the column term v = G @ (dk Wk_h b_Q,h) rides the exp bias.
  g1/be1 fold into W_O / a bias row; g2/be2 fold into W1/b1 for the MLP.
  B_bias is pre-transposed + fp8-cast on host and added into PSUM by the PE
  via identity matmuls (DoubleRow fp8: 0.5 cyc/row).
  LayerNorm rstd uses exp(-0.5*ln(x)) so the ACT engine never leaves the
  {exp, ln, relu} activation table (no table reloads).
  All transposes ride the DMA XBAR (A^T from DRAM, z->mhcT, oz->oT, r2T back).

Self-contained: hardcodes all shapes; imports only the in-container concourse
stack.
"""

import sys

sys.path.insert(0, "/opt/trn_rl_repo")

import numpy as np
import ml_dtypes
from contextlib import ExitStack

import concourse.bass as bass
import concourse.tile as tile
from concourse import bacc
from concourse import mybir
from concourse.bass_utils import run_bass_kernel_spmd

F32 = mybir.dt.float32
BF16 = mybir.dt.bfloat16
FP8 = mybir.dt.float8e4
AF = mybir.ActivationFunctionType
OP = mybir.AluOpType
AX = mybir.AxisListType
PM = mybir.MatmulPerfMode

B = 8
N = 1024
D = 128
HEADS = 8
DV = 128
HD = HEADS * DV  # 1024
P = 128
NT = N // P  # 8 tiles of 128 rows
DK = 1.0 / float(np.sqrt(np.float32(D)))
EPS = 1e-5

USE_DOUBLEROW = True
DEBUG_TAPS = False

_prog_cache = {}


def _bcast_load(nc, dst, src):
    """DMA-load 1D DRAM vector src [W] replicated across all P partitions of
    dst [P, W] (issued on the ACT HWDGE queue)."""
    rep = bass.AP(tensor=src.tensor, offset=src.offset, ap=[[0, P]] + list(src.ap))
    nc.scalar.dma_start(out=dst, in_=rep)


U32 = mybir.dt.uint32
_RSQRT_MAGIC = 0x5F3759DF


def _rsqrt_pool(nc, pool, out_ap, in_ap, w, tag, cns):
    """Pool-engine rsqrt via tensor_tensor-only ops (Pool rejects
    TensorScalarPtr). cns = dict of const tiles sliced to [P, w]:
    one (u32 1), magic, c15 (1.5), cm05 (-0.5)."""
    eng = nc.gpsimd
    y = pool.tile([P, w], F32, name=f"rp_y{tag}", tag=f"rp_y{w}")
    ib = pool.tile([P, w], U32, name=f"rp_i{tag}", tag=f"rp_i{w}")
    # seed on DVE (Pool shifts require u64 outputs); iterations on Pool
    nc.vector.tensor_scalar(
        out=ib, in0=in_ap.bitcast(U32), scalar1=1, scalar2=None,
        op0=OP.logical_shift_right)
    nc.vector.tensor_tensor(out=y.bitcast(U32), in0=cns["magic"], in1=ib,
                            op=OP.subtract)
    for it in range(2):
        a = pool.tile([P, w], F32, name=f"rp_a{tag}_{it}", tag=f"rp_a{w}")
        eng.tensor_tensor(out=a, in0=y, in1=y, op=OP.mult)
        eng.tensor_tensor(out=a, in0=a, in1=in_ap, op=OP.mult)
        eng.tensor_tensor(out=a, in0=a, in1=cns["cm05"], op=OP.mult)
        eng.tensor_tensor(out=a, in0=a, in1=cns["c15"], op=OP.add)
        dst = out_ap if it == 1 else y
        eng.tensor_tensor(out=dst, in0=y, in1=a, op=OP.mult)


def _rsqrt(nc, eng, pool, out_ap, in_ap, w, tag, magic):
    """out = 1/sqrt(in) elementwise on [P, w] f32 via bit-hack seed + 2 Newton
    iterations. Runs entirely on `eng` (vector or gpsimd) — avoids the ACT
    table thrash that Sqrt/Ln would cause next to Exp."""
    y = pool.tile([P, w], F32, name=f"rq_y{tag}", tag=f"rq_y{w}")
    ib = pool.tile([P, w], U32, name=f"rq_i{tag}", tag=f"rq_i{w}")
    eng.tensor_scalar(
        out=ib, in0=in_ap.bitcast(U32), scalar1=1, scalar2=None,
        op0=OP.logical_shift_right)
    eng.tensor_tensor(out=y.bitcast(U32), in0=magic, in1=ib, op=OP.subtract)
    for it in range(2):
        a = pool.tile([P, w], F32, name=f"rq_a{tag}_{it}", tag=f"rq_a{w}")
        eng.tensor_tensor(out=a, in0=y, in1=y, op=OP.mult)
        eng.tensor_tensor(out=a, in0=a, in1=in_ap, op=OP.mult)
        eng.tensor_scalar(
            out=a, in0=a, scalar1=-0.5, scalar2=1.5, op0=OP.mult, op1=OP.add)
        dst = out_ap if it == 1 else y
        eng.tensor_tensor(out=dst, in0=y, in1=a, op=OP.mult)


def _build_program():
    nc = bacc.Bacc(None)

    h_in = nc.declare_dram_parameter("h", [N, D], F32, isOutput=False)
    a16_in = nc.declare_dram_parameter("a16", [N, N], BF16, isOutput=False)
    bt_in = nc.declare_dram_parameter("bt", [HEADS, N, N], FP8, isOutput=False)
    wp32_in = nc.declare_dram_parameter("wp32", [P, 1666], F32, isOutput=False)
    wp16_in = nc.declare_dram_parameter("wp16", [P, 3464], BF16, isOutput=False)
    out_dram = nc.declare_dram_parameter("out", [N, D], F32, isOutput=True)
    taps = {}
    if DEBUG_TAPS:
        taps["t_disrow"] = nc.declare_dram_parameter(
            "t_disrow", [P, N], F32, isOutput=True)
        taps["t_GT"] = nc.declare_dram_parameter(
            "t_GT", [P, N], BF16, isOutput=True)
        taps["t_aT"] = nc.declare_dram_parameter(
            "t_aT", [P, NT, N], BF16, isOutput=True)
        taps["t_hnS"] = nc.declare_dram_parameter(
            "t_hnS", [P, NT, D], BF16, isOutput=True)
        taps["t_vna"] = nc.declare_dram_parameter(
            "t_vna", [P, NT, HEADS, DV + 1], BF16, isOutput=True)
        taps["t_et0"] = nc.declare_dram_parameter(
            "t_et0", [P, NT, N], BF16, isOutput=True)
        taps["t_mhcT"] = nc.declare_dram_parameter(
            "t_mhcT", [P, HEADS, N], BF16, isOutput=True)
        taps["t_orow"] = nc.declare_dram_parameter(
            "t_orow", [P, NT, D], F32, isOutput=True)
        taps["t_r2n"] = nc.declare_dram_parameter(
            "t_r2n", [P, NT, D], BF16, isOutput=True)
    disr = nc.dram_tensor("disr", [N], F32, kind="Internal")

    with tile.TileContext(nc) as tc, ExitStack() as ctx:
        consts = ctx.enter_context(tc.tile_pool(name="consts", bufs=1))
        persist = ctx.enter_context(tc.tile_pool(name="persist", bufs=1))
        small = ctx.enter_context(tc.tile_pool(name="small", bufs=12))

        # ---- constants -------------------------------------------------
        # diag-fix masks in bf16: omi = 1 - I, identb = I
        omi = consts.tile([P, P], BF16)
        nc.gpsimd.memset(omi, 1.0)
        nc.gpsimd.affine_select(
            out=omi, in_=omi, compare_op=OP.not_equal, fill=0.0,
            base=0, pattern=[[-1, P]], channel_multiplier=1)
        identb = consts.tile([P, P], BF16)
        nc.gpsimd.memset(identb, 0.0)
        nc.gpsimd.affine_select(
            out=identb, in_=identb, compare_op=OP.not_equal, fill=1.0,
            base=0, pattern=[[-1, P]], channel_multiplier=1)
        if USE_DOUBLEROW:
            # [I | 0] and [0 | I] fp8 stationaries for DoubleRow bias-adds
            id2a = consts.tile([P, 2, P], FP8)
            nc.gpsimd.memset(id2a, 0.0)
            nc.vector.tensor_copy(out=id2a[:, 0, :], in_=identb)
            id2b = consts.tile([P, 2, P], FP8)
            nc.gpsimd.memset(id2b, 0.0)
            nc.vector.tensor_copy(out=id2b[:, 1, :], in_=identb)
        else:
            id8 = consts.tile([P, P], FP8)
            nc.vector.tensor_copy(out=id8, in_=identb)
        ones1 = consts.tile([1, P], BF16)
        nc.vector.memset(ones1, 1.0)
        magic8 = consts.tile([P, NT], U32)
        nc.vector.memset(magic8, _RSQRT_MAGIC)
        one8 = consts.tile([P, NT], U32)
        nc.vector.memset(one8, 1)
        c158 = consts.tile([P, NT], F32)
        nc.vector.memset(c158, 1.5)
        cm058 = consts.tile([P, NT], F32)
        nc.vector.memset(cm058, -0.5)
        onescol = consts.tile([P, 1], BF16)
        nc.vector.memset(onescol, 1.0)

        # ---- phase A/B: H + A + packed-weight loads ---------------------
        h_sb = persist.tile([P, NT, D], F32, tag="h")
        nc.scalar.dma_start(out=h_sb, in_=h_in.rearrange("(t p) d -> p t d", p=P))
        wp32 = consts.tile([P, 1666], F32)
        nc.scalar.dma_start(out=wp32, in_=wp32_in[:, :])
        wp16 = consts.tile([P, 3464], BF16)
        nc.scalar.dma_start(out=wp16, in_=wp16_in[:, :])
        g0b = wp32[:, 0:128]
        be0b = wp32[:, 128:256]
        g2b = wp32[:, 256:384]
        g3b = wp32[:, 384:512]
        be23b = wp32[:, 512:640]
        bvb = wp32[:, 640:1664]
        b1p = wp32[:, 1664:1665]
        b2p = wp32[:, 1665:1666]
        mh_sb = wp16[:, 0:1024].rearrange("p (hh d) -> p hh d", hh=HEADS)
        wv_sb = wp16[:, 1024:2048]
        wo_sb = wp16[:, 2048:3072].rearrange("p (hh d) -> p hh d", hh=HEADS)
        w1_sb = wp16[:, 3072:3200]
        w2_sb = wp16[:, 3200:3328]
        qb_sb = wp16[:, 3328:3336]
        bo_row = wp16[0:1, 3336:3464]

        # A loads: natural chunks (scoped) feed DVE rowsums; A^T via XBAR
        # into 8 SEPARATE tiles (a shared tile serializes the XBARs against
        # the diag-fix RMWs through tile-level dep tracking).
        rs_all = small.tile([P, NT], F32, tag="rs_all")
        mvH = small.tile([P, NT, 2], F32, tag="mvH")
        anat_ctx = tc.tile_pool(name="anatp", bufs=1)
        anatp = anat_ctx.__enter__()
        anat = anatp.tile([P, NT, N], BF16, tag="anat")
        aTt = persist.tile([P, NT, N], BF16, tag="aTt")  # [m-chunk, n]
        aT = [aTt[:, i, :] for i in range(NT)]
        psT_ctx = tc.tile_pool(name="psT", bufs=4, space=bass.MemorySpace.PSUM)
        psT = psT_ctx.__enter__()
        for j in range(NT):
            an = anat[:, j, :]
            nc.sync.dma_start(out=an, in_=a16_in[j * P:(j + 1) * P, :])
            db = anat[:, j, j * P:(j + 1) * P]
            nc.gpsimd.tensor_tensor(out=db, in0=db, in1=omi, op=OP.mult)
            nc.gpsimd.tensor_tensor(out=db, in0=db, in1=identb, op=OP.add)
            nc.vector.reduce_sum(
                out=rs_all[:, j:j + 1], in_=an, axis=AX.X)
            # transpose the 8 blocks of this natural chunk on the (idle) PE;
            # one strided 512-wide copy per 4 blocks, alternating ACT/DVE
            for g in range(2):
                pt = psT.tile([P, 4, P], BF16, tag="pt", name=f"pt{j}_{g}")
                for ii in range(4):
                    i = g * 4 + ii
                    nc.tensor.transpose(
                        pt[:, ii, :], anat[:, j, i * P:(i + 1) * P], identb)
                dst = aTt[:, g * 4:(g + 1) * 4, j * P:(j + 1) * P]
                nc.scalar.copy(out=dst, in_=pt)
        psT_ctx.__exit__(None, None, None)
        for j in range(NT):
            # H LN stats (independent of A)
            s6 = small.tile([P, 6], F32, tag="s6h", name=f"s6h{j}")
            nc.vector.bn_stats(out=s6, in_=h_sb[:, j, :])
            nc.vector.bn_aggr(out=mvH[:, j, :], in_=s6)

        # dis = rsqrt(max(rowsum, 1))  [P, NT]
        dmax = small.tile([P, NT], F32, tag="dmax")
        nc.vector.tensor_scalar_max(out=dmax, in0=rs_all, scalar1=1.0)
        dis_sb = small.tile([P, NT], F32, tag="dis")
        _rsqrt(nc, nc.vector, small, dis_sb, dmax, NT, "dis", magic8)
        anat_ctx.__exit__(None, None, None)

        # HnS = dis * LN(H):  rstd2 = rstdH*dis
        tvH = small.tile([P, NT], F32, tag="tvH")
        nc.vector.tensor_scalar_add(
            out=tvH, in0=mvH[:, :, 1], scalar1=EPS)
        rstdH = small.tile([P, NT], F32, tag="rstdH")
        _rsqrt(nc, nc.vector, small, rstdH, tvH, NT, "H", magic8)
        rstd2 = small.tile([P, NT], F32, tag="rstd2")
        nc.vector.tensor_tensor(out=rstd2, in0=rstdH, in1=dis_sb, op=OP.mult)

        # disrow [P, N] broadcast of dis in n-order via DRAM round-trip
        nc.sync.dma_start(out=disr.rearrange("(t p) -> p t", p=P), in_=dis_sb)
        disrow = persist.tile([P, N], F32, tag="disrow")
        rep = bass.AP(
            tensor=disr[:].tensor, offset=disr[:].offset,
            ap=[[0, P]] + list(disr[:].ap))
        nc.sync.dma_start(out=disrow, in_=rep)

        hnS = persist.tile([P, NT, D], BF16, tag="hnS")
        for j in range(NT):
            zh = small.tile([P, D], F32, tag="zh")
            nc.vector.tensor_scalar(
                out=zh, in0=h_sb[:, j, :], scalar1=mvH[:, j, 0:1],
                scalar2=rstd2[:, j:j + 1], op0=OP.subtract, op1=OP.mult)
            hn1 = small.tile([P, D], BF16, tag="hn1")
            nc.gpsimd.tensor_tensor(out=hn1, in0=zh, in1=g0b, op=OP.mult)
            be0S = small.tile([P, D], BF16, tag="be0S")
            nc.gpsimd.tensor_scalar_mul(
                out=be0S, in0=be0b, scalar1=dis_sb[:, j:j + 1])
            nc.gpsimd.tensor_tensor(
                out=hnS[:, j, :], in0=hn1, in1=be0S, op=OP.add)

        # ---- G^T = dis_n * (HnS^T @ Ahat^T)  [d, n] bf16 ---------------
        GT = persist.tile([P, N], BF16, tag="GT")
        with tc.tile_pool(name="psG", bufs=2, space=bass.MemorySpace.PSUM) as psG:
            for c in range(2):
                g0t = psG.tile([P, 512], F32, tag="g0t")
                for j in range(NT):
                    nc.tensor.matmul(
                        g0t, hnS[:, j, :], aT[j][:, c * 512:(c + 1) * 512],
                        start=(j == 0), stop=(j == NT - 1))
                nc.vector.tensor_tensor(
                    out=GT[:, c * 512:(c + 1) * 512], in0=g0t,
                    in1=disrow[:, c * 512:(c + 1) * 512], op=OP.mult)

        if DEBUG_TAPS:
            nc.sync.dma_start(out=taps["t_disrow"][:, :], in_=disrow)
            nc.sync.dma_start(out=taps["t_GT"][:, :], in_=GT)
            for _i in range(NT):
                nc.sync.dma_start(out=taps["t_aT"][:, _i, :], in_=aT[_i])
            nc.sync.dma_start(out=taps["t_hnS"][:, :, :], in_=hnS)

        # ---- V = GT^T Wv + bv (vna with ones column), v = GT^T qb ------
        vna = persist.tile([P, NT, HEADS, DV], BF16, tag="vna")
        v_sb = persist.tile([P, NT, HEADS], F32, tag="v_sb")
        with tc.tile_pool(name="psV", bufs=2, space=bass.MemorySpace.PSUM) as psV, \
             tc.tile_pool(name="psv", bufs=2, space=bass.MemorySpace.PSUM) as psv:
            for i in range(NT):
                vq = psv.tile([P, HEADS], F32, tag="vq")
                nc.tensor.matmul(
                    vq, GT[:, i * P:(i + 1) * P], qb_sb, start=True, stop=True)
                nc.vector.tensor_copy(out=v_sb[:, i, :], in_=vq)
                for c in range(2):
                    vp = psV.tile([P, 512], F32, tag="vp")
                    nc.tensor.matmul(
                        vp, GT[:, i * P:(i + 1) * P],
                        wv_sb[:, c * 512:(c + 1) * 512], start=True, stop=True)
                    nc.vector.tensor_tensor(
                        out=vna[:, i, c * 4:(c + 1) * 4, 0:DV],
                        in0=vp.rearrange("p (a b) -> p a b", a=4),
                        in1=bvb[:, c * 512:(c + 1) * 512].rearrange(
                            "p (a b) -> p a b", a=4),
                        op=OP.add)

        # ---- phase D: attention, software-pipelined over heads ---------
        # PE stream per head: [P_h matmuls] -> [PV of head h-1] -> [S+B of h],
        # so the tensor engine never waits for the ACT exp chain.
        mhcT = persist.tile([P, HEADS, N], BF16, tag="mhcT")
        bt_tiles = {}
        et_tiles = {}
        psb_tiles = {}
        BFLAT = NT * N + 512  # flat per-head B^T + zero pad for DoubleRow view
        with tc.tile_pool(name="btp", bufs=3) as btp, \
             tc.tile_pool(name="pp", bufs=2) as ppool, \
             tc.tile_pool(name="etp", bufs=2) as etp, \
             tc.tile_pool(name="zrp", bufs=6) as zrp, \
             tc.tile_pool(name="psS", bufs=3, space=bass.MemorySpace.PSUM) as psS, \
             tc.tile_pool(name="psM", bufs=2, space=bass.MemorySpace.PSUM) as psM:

            def load_bt(h):
                t = btp.tile([P, BFLAT], FP8, tag="bt", name=f"bt{h}")
                nc.gpsimd.memset(t[:, NT * N:], 0.0)
                nc.sync.dma_start(
                    out=t[:, 0:NT * N].rearrange("p (t n) -> p t n", t=NT),
                    in_=bt_in[h].rearrange("(t p) n -> p t n", p=P))
                bt_tiles[h] = t

            def prep_head(h):
                pp = psS.tile([P, N], F32, tag="st", name=f"pp{h}")
                for c in range(2):
                    nc.tensor.matmul(
                        pp[:, c * 512:(c + 1) * 512], mh_sb[:, h, :],
                        GT[:, c * 512:(c + 1) * 512], start=True, stop=True)
                p_sb = ppool.tile([P, N], BF16, tag="p_sb", name=f"psb{h}")
                for c in range(2):
                    # head 0/1 copies on ACT (idle at startup; DVE is busy
                    # with the V-phase adds then)
                    if h < 2:
                        nc.scalar.copy(
                            out=p_sb[:, c * 512:(c + 1) * 512],
                            in_=pp[:, c * 512:(c + 1) * 512])
                    else:
                        nc.vector.tensor_copy(
                            out=p_sb[:, c * 512:(c + 1) * 512],
                            in_=pp[:, c * 512:(c + 1) * 512])
                psb_tiles[h] = p_sb

            def s_loop(h, j0, j1):
                if j0 == 0:
                    et_tiles[h] = etp.tile(
                        [P, NT, N], BF16, tag="et", name=f"et{h}")
                et = et_tiles[h]
                btt = bt_tiles[h]
                p_sb = psb_tiles[h]
                for j in range(j0, j1):
                    st = psS.tile([P, N], F32, tag="st", name=f"st{h}_{j}")
                    for c in range(2):
                        nc.tensor.matmul(
                            st[:, c * 512:(c + 1) * 512],
                            p_sb[:, j * P:(j + 1) * P],
                            GT[:, c * 512:(c + 1) * 512],
                            start=True, stop=False)
                    for c in range(2):
                        if USE_DOUBLEROW:
                            rv = btt[:, j * N + c * 512:
                                     j * N + c * 512 + 1024].rearrange(
                                "p (a b) -> p a b", a=2)
                            nc.tensor.matmul(
                                st[:, c * 512:(c + 1) * 512], id2a, rv,
                                start=False, stop=True,
                                perf_mode=PM.DoubleRow, skip_group_check=True)
                        else:
                            nc.tensor.matmul(
                                st[:, c * 512:(c + 1) * 512], id8,
                                btt[:, j * N + c * 512:j * N + (c + 1) * 512],
                                start=False, stop=True, skip_group_check=True)
                    nc.scalar.activation(
                        out=et[:, j, :], in_=st, func=AF.Exp,
                        bias=v_sb[:, j, h:h + 1])
                if DEBUG_TAPS and h == 0:
                    nc.sync.dma_start(out=taps["t_et0"][:, :, :], in_=et)

            def pv_half(h, half):
                et = et_tiles[h]
                pm = psM.tile([P, 4, P], F32, tag="pm", name=f"pm{h}_{half}")
                mvv = small.tile([P, 4, 2], F32, tag="mvv",
                                 name=f"mvv{h}_{half}")
                for q in range(4):
                    i = half * 4 + q
                    for j in range(NT):
                        nc.tensor.matmul(
                            pm[:, q, :],
                            et[:, j, i * P:(i + 1) * P],
                            vna[:, j, h, :],
                            start=(j == 0), stop=(j == NT - 1))
                    s6 = small.tile([P, 6], F32, tag="s6a",
                                    name=f"s6a{h}_{half}_{q}")
                    nc.vector.bn_stats(out=s6, in_=pm[:, q, :])
                    nc.vector.bn_aggr(out=mvv[:, q, :], in_=s6)
                t3 = small.tile([P, 4], F32, tag="t3", name=f"t3{h}_{half}")
                nc.vector.tensor_scalar_add(
                    out=t3, in0=mvv[:, :, 1], scalar1=EPS)
                rstd = small.tile([P, 4], F32, tag="rstda",
                                  name=f"rsd{h}_{half}")
                _rsqrt_pool(nc, small, rstd, t3, 4, f"a{h}_{half}", {
                    "one": one8[:, 0:4], "magic": magic8[:, 0:4],
                    "c15": c158[:, 0:4], "cm05": cm058[:, 0:4]})
                for q in range(4):
                    i = half * 4 + q
                    zrow = zrp.tile([P, DV], BF16, tag="zrow",
                                    name=f"z{h}_{half}_{q}")
                    nc.vector.tensor_scalar(
                        out=zrow, in0=pm[:, q, :],
                        scalar1=mvv[:, q, 0:1],
                        scalar2=rstd[:, q:q + 1],
                        op0=OP.subtract, op1=OP.mult)
                    nc.sync.dma_start_transpose(
                        out=mhcT[:, h, i * P:(i + 1) * P], in_=zrow)

            load_bt(0)
            load_bt(1)
            prep_head(0)
            prep_head(1)
            s_loop(0, 0, NT)
            for h in range(1, HEADS):
                if h + 1 < HEADS:
                    load_bt(h + 1)
                s_loop(h, 0, NT)
                if h + 1 < HEADS:
                    prep_head(h + 1)
                pv_half(h - 1, 0)
                pv_half(h - 1, 1)
            pv_half(HEADS - 1, 0)
            pv_half(HEADS - 1, 1)

        # ---- phase E: output projection + MLP --------------------------
        orow_sb = persist.tile([P, NT, D], F32, tag="orow")
        mvO = small.tile([P, NT, 2], F32, tag="mvO")
        with tc.tile_pool(name="psE", bufs=2, space=bass.MemorySpace.PSUM) as psE:
            for i in range(NT):
                op = psE.tile([P, D], F32, tag="op")
                for hh in range(HEADS):
                    nc.tensor.matmul(
                        op, mhcT[:, hh, i * P:(i + 1) * P], wo_sb[:, hh, :],
                        start=(hh == 0), stop=False)
                nc.tensor.matmul(
                    op, ones1, bo_row, start=False, stop=True,
                    skip_group_check=True)
                nc.vector.tensor_tensor(
                    out=orow_sb[:, i, :], in0=op, in1=h_sb[:, i, :], op=OP.add)
                s6 = small.tile([P, 6], F32, tag="s6o")
                nc.vector.bn_stats(out=s6, in_=orow_sb[:, i, :])
                nc.vector.bn_aggr(out=mvO[:, i, :], in_=s6)
        if DEBUG_TAPS:
            nc.sync.dma_start(out=taps["t_orow"][:, :, :], in_=orow_sb)
        tvO = small.tile([P, NT], F32, tag="tvO")
        nc.vector.tensor_scalar_add(
            out=tvO, in0=mvO[:, :, 1], scalar1=EPS)
        rstdO = small.tile([P, NT], F32, tag="rstdO")
        _rsqrt(nc, nc.vector, small, rstdO, tvO, NT, "O", magic8)
        oz = persist.tile([P, NT, D], BF16, tag="oz")
        oT = persist.tile([P, N], BF16, tag="oT")
        f1_sb = persist.tile([P, NT, D], F32, tag="f1_sb")
        psE1_ctx = tc.tile_pool(name="psE1", bufs=2, space=bass.MemorySpace.PSUM)
        psE1 = psE1_ctx.__enter__()
        for g in range(2):
            pt = psE1.tile([P, 4, P], BF16, tag="eT", name=f"ozT{g}")
            for ii in range(4):
                i = g * 4 + ii
                nc.vector.tensor_scalar(
                    out=oz[:, i, :], in0=orow_sb[:, i, :],
                    scalar1=mvO[:, i, 0:1],
                    scalar2=rstdO[:, i:i + 1], op0=OP.subtract, op1=OP.mult)
                nc.tensor.transpose(pt[:, ii, :], oz[:, i, :], identb)
                # f1 = oz*g2 + be23 computed early on Pool, overlapping the MLP
                nc.gpsimd.tensor_tensor(
                    out=f1_sb[:, i, :], in0=oz[:, i, :], in1=g2b, op=OP.mult)
                nc.gpsimd.tensor_tensor(
                    out=f1_sb[:, i, :], in0=f1_sb[:, i, :], in1=be23b,
                    op=OP.add)
            nc.scalar.copy(
                out=oT[:, g * 512:(g + 1) * 512],
                in_=pt.rearrange("p a b -> p (a b)"))

        r1T = persist.tile([P, N], BF16, tag="r1T")
        r2T = persist.tile([P, N], BF16, tag="r2T")
        with tc.tile_pool(name="psE2", bufs=2, space=bass.MemorySpace.PSUM) as psE2:
            for c in range(2):
                ps = psE2.tile([P, 512], F32, tag="ps2")
                nc.tensor.matmul(
                    ps, w1_sb, oT[:, c * 512:(c + 1) * 512], start=True, stop=True)
                nc.scalar.activation(
                    out=r1T[:, c * 512:(c + 1) * 512], in_=ps, func=AF.Relu,
                    bias=b1p)
            for c in range(2):
                ps = psE2.tile([P, 512], F32, tag="ps2")
                nc.tensor.matmul(
                    ps, w2_sb, r1T[:, c * 512:(c + 1) * 512], start=True, stop=True)
                nc.scalar.activation(
                    out=r2T[:, c * 512:(c + 1) * 512], in_=ps, func=AF.Relu,
                    bias=b2p)

        mvR = small.tile([P, NT, 2], F32, tag="mvR")
        r2p = []
        for g in range(2):
            pt = psE1.tile([P, 4, P], BF16, tag="eT", name=f"r2T{g}")
            r2p.append(pt)
            for ii in range(4):
                i = g * 4 + ii
                nc.tensor.transpose(
                    pt[:, ii, :], r2T[:, i * P:(i + 1) * P], identb)
                s6 = small.tile([P, 6], F32, tag="s6r", name=f"s6r{i}")
                nc.vector.bn_stats(out=s6, in_=pt[:, ii, :])
                nc.vector.bn_aggr(out=mvR[:, i, :], in_=s6)
        if DEBUG_TAPS:
            for _g in range(2):
                nc.sync.dma_start(
                    out=taps["t_r2n"][:, _g * 4:(_g + 1) * 4, :], in_=r2p[_g])
        tvR = small.tile([P, NT], F32, tag="tvR")
        nc.vector.tensor_scalar_add(
            out=tvR, in0=mvR[:, :, 1], scalar1=EPS)
        rstdR = small.tile([P, NT], F32, tag="rstdR")
        _rsqrt(nc, nc.vector, small, rstdR, tvR, NT, "R", magic8)

        out_sb = persist.tile([P, NT, D], F32, tag="osb")
        for i in range(NT):
            zr = small.tile([P, D], F32, tag="zr", name=f"zr{i}")
            nc.vector.tensor_scalar(
                out=zr, in0=r2p[i // 4][:, i % 4, :], scalar1=mvR[:, i, 0:1],
                scalar2=rstdR[:, i:i + 1], op0=OP.subtract, op1=OP.mult)
            f2 = small.tile([P, D], F32, tag="f2", name=f"f2{i}")
            nc.vector.tensor_tensor(out=f2, in0=zr, in1=g3b, op=OP.mult)
            nc.vector.tensor_tensor(
                out=out_sb[:, i, :], in0=f1_sb[:, i, :], in1=f2, op=OP.add)
        psE1_ctx.__exit__(None, None, None)
        for g in range(2):
            nc.sync.dma_start(
                out=out_dram.rearrange(
                    "(t p) d -> p t d", p=P)[:, g * 4:(g + 1) * 4, :],
                in_=out_sb[:, g * 4:(g + 1) * 4, :])

    nc.compile()
    return nc


def _get_program():
    if "nc" not in _prog_cache:
        _prog_cache["nc"] = _build_program()
    return _prog_cache["nc"]


def kernel(**inputs):
    nc = _get_program()
    f32 = np.float32
    bf16 = ml_dtypes.bfloat16
    fp8 = ml_dtypes.float8_e4m3

    H = np.asarray(inputs["H"], dtype=f32)
    A = np.asarray(inputs["A"], dtype=f32)
    WQ = np.asarray(inputs["W_Q"], dtype=f32)
    WK = np.asarray(inputs["W_K"], dtype=f32)
    WV = np.asarray(inputs["W_V"], dtype=f32)
    WO = np.asarray(inputs["W_O"], dtype=f32)
    bQ = np.asarray(inputs["b_Q"], dtype=f32)
    g1 = np.asarray(inputs["g1"], dtype=f32)
    be1 = np.asarray(inputs["be1"], dtype=f32)
    g2 = np.asarray(inputs["g2"], dtype=f32)
    be2 = np.asarray(inputs["be2"], dtype=f32)
    W1 = np.asarray(inputs["W1"], dtype=f32)
    b1 = np.asarray(inputs["b1"], dtype=f32)

    mh = np.stack([
        DK * WK[:, h * DV:(h + 1) * DV] @ WQ[:, h * DV:(h + 1) * DV].T
        for h in range(HEADS)
    ]).astype(bf16)
    qb = np.stack([
        DK * WK[:, h * DV:(h + 1) * DV] @ bQ[h * DV:(h + 1) * DV]
        for h in range(HEADS)
    ], axis=1).astype(bf16)  # [D, HEADS]
    WOp = (g1[:, None] * WO.reshape(HEADS, DV, D)).astype(bf16)  # [H, DV, D]
    bo = (be1 @ WO.reshape(HEADS, DV, D).sum(0)).reshape(1, D).astype(bf16)
    W1p = (g2[:, None] * W1).astype(bf16)
    b1p = (be2 @ W1 + b1).reshape(D, 1).astype(f32)
    be23 = (be2 + np.asarray(inputs["be3"], dtype=f32)).astype(f32)

    BT = np.ascontiguousarray(
        np.asarray(inputs["B_bias"], dtype=f32).transpose(0, 2, 1)).astype(fp8)

    wp32 = np.zeros((P, 1666), f32)
    wp32[:, 0:128] = np.asarray(inputs["g0"], dtype=f32)[None, :]
    wp32[:, 128:256] = np.asarray(inputs["be0"], dtype=f32)[None, :]
    wp32[:, 256:384] = g2[None, :]
    wp32[:, 384:512] = np.asarray(inputs["g3"], dtype=f32)[None, :]
    wp32[:, 512:640] = be23[None, :]
    wp32[:, 640:1664] = np.asarray(inputs["b_V"], dtype=f32)[None, :]
    wp32[:, 1664] = b1p[:, 0]
    wp32[:, 1665] = np.asarray(inputs["b2"], dtype=f32)
    wp16 = np.zeros((P, 3464), bf16)
    wp16[:, 0:1024] = mh.transpose(1, 0, 2).reshape(P, HEADS * D)
    wp16[:, 1024:2048] = WV.astype(bf16)
    wp16[:, 2048:3072] = WOp.transpose(1, 0, 2).reshape(P, HEADS * D)
    wp16[:, 3072:3200] = W1p
    wp16[:, 3200:3328] = np.asarray(inputs["W2"], dtype=f32).astype(bf16)
    wp16[:, 3328:3336] = qb
    wp16[:, 3336:3464] = bo[None, 0, :]
    base = {
        "bt": BT,
        "wp32": wp32,
        "wp16": np.ascontiguousarray(wp16),
    }

    in_maps = []
    for c in range(B):
        m = dict(base)
        m["h"] = np.ascontiguousarray(H[c])
        m["a16"] = np.ascontiguousarray(A[c]).astype(bf16)
        in_maps.append(m)

    res = run_bass_kernel_spmd(nc, in_maps, list(range(B)))
    if DEBUG_TAPS:
        _prog_cache["taps"] = res.results
    out = np.stack([res.results[c]["out"] for c in range(B)], axis=0)
    return out.astype(np.float32)


if __name__ == "__main__":
    _get_program()
    print("program built ok")


# revision 40
# speedup vs baseline: 1.1929x; 1.0232x over previous
"""Trainium2 Bass kernel for nn_GRIC_31550829756424 (GCN-attention block).

Data-parallel over batch: 8 batches -> 8 NeuronCores, one full batch per core.

Algebraic restructure vs the straightforward lowering:
  Q = adj @ (Hn Wq) = (adj Hn) Wq  ==>  G = adj_norm @ Hn computed ONCE.
  S^T_h = G M2_h G^T with M2_h = dk*Wk_h Wq_h^T folded on host, so scores need
  no Q/K materialization. Row-constant score-bias terms cancel in softmax and
  are dropped; the column term v = G @ (dk Wk_h b_Q,h) rides the exp bias.
  g1/be1 fold into W_O / a bias row; g2/be2 fold into W1/b1 for the MLP.
  B_bias is pre-transposed + fp8-cast on host and added into PSUM by the PE
  via identity matmuls (DoubleRow fp8: 0.5 cyc/row).
  LayerNorm rstd uses exp(-0.5*ln(x)) so the ACT engine never leaves the
  {exp, ln, relu} activation table (no table reloads).
  All transposes ride the DMA XBAR (A^T from DRAM, z->mhcT, oz->oT, r2T back).

Self-contained: hardcodes all shapes; imports only the in-container concourse
stack.
"""

import sys

sys.path.insert(0, "/opt/trn_rl_repo")

import numpy as np
import ml_dtypes
from contextlib import ExitStack

import concourse.bass as bass
import concourse.tile as tile
from concourse import bacc
from concourse import mybir
from concourse.bass_utils import run_bass_kernel_spmd

F32 = mybir.dt.float32
BF16 = mybir.dt.bfloat16
FP8 = mybir.dt.float8e4
AF = mybir.ActivationFunctionType
OP = mybir.AluOpType
AX = mybir.AxisListType
PM = mybir.MatmulPerfMode

B = 8
N = 1024
D = 128
HEADS = 8
DV = 128
HD = HEADS * DV  # 1024
P = 128
NT = N // P  # 8 tiles of 128 rows
DK = 1.0 / float(np.sqrt(np.float32(D)))
EPS = 1e-5

USE_DOUBLEROW = True
DEBUG_TAPS = False

_prog_cache = {}


def _bcast_load(nc, dst, src):
    """DMA-load 1D DRAM vector src [W] replicated across all P partitions of
    dst [P, W] (issued on the ACT HWDGE queue)."""
    rep = bass.AP(tensor=src.tensor, offset=src.offset, ap=[[0, P]] + list(src.ap))
    nc.scalar.dma_start(out=dst, in_=rep)


U32 = mybir.dt.uint32
_RSQRT_MAGIC = 0x5F3759DF


def _rsqrt_pool(nc, pool, out_ap, in_ap, w, tag, cns):
    """Pool-engine rsqrt via tensor_tensor-only ops (Pool rejects
    TensorScalarPtr). cns = dict of const tiles sliced to [P, w]:
    one (u32 1), magic, c15 (1.5), cm05 (-0.5)."""
    eng = nc.gpsimd
    y = pool.tile([P, w], F32, name=f"rp_y{tag}", tag=f"rp_y{w}")
    ib = pool.tile([P, w], U32, name=f"rp_i{tag}", tag=f"rp_i{w}")
    # seed on DVE (Pool shifts require u64 outputs); iterations on Pool
    nc.vector.tensor_scalar(
        out=ib, in0=in_ap.bitcast(U32), scalar1=1, scalar2=None,
        op0=OP.logical_shift_right)
    nc.vector.tensor_tensor(out=y.bitcast(U32), in0=cns["magic"], in1=ib,
                            op=OP.subtract)
    for it in range(2):
        a = pool.tile([P, w], F32, name=f"rp_a{tag}_{it}", tag=f"rp_a{w}")
        eng.tensor_tensor(out=a, in0=y, in1=y, op=OP.mult)
        eng.tensor_tensor(out=a, in0=a, in1=in_ap, op=OP.mult)
        eng.tensor_tensor(out=a, in0=a, in1=cns["cm05"], op=OP.mult)
        eng.tensor_tensor(out=a, in0=a, in1=cns["c15"], op=OP.add)
        dst = out_ap if it == 1 else y
        eng.tensor_tensor(out=dst, in0=y, in1=a, op=OP.mult)


def _rsqrt(nc, eng, pool, out_ap, in_ap, w, tag, magic):
    """out = 1/sqrt(in) elementwise on [P, w] f32 via bit-hack seed + 2 Newton
    iterations. Runs entirely on `eng` (vector or gpsimd) — avoids the ACT
    table thrash that Sqrt/Ln would cause next to Exp."""
    y = pool.tile([P, w], F32, name=f"rq_y{tag}", tag=f"rq_y{w}")
    ib = pool.tile([P, w], U32, name=f"rq_i{tag}", tag=f"rq_i{w}")
    eng.tensor_scalar(
        out=ib, in0=in_ap.bitcast(U32), scalar1=1, scalar2=None,
        op0=OP.logical_shift_right)
    eng.tensor_tensor(out=y.bitcast(U32), in0=magic, in1=ib, op=OP.subtract)
    for it in range(2):
        a = pool.tile([P, w], F32, name=f"rq_a{tag}_{it}", tag=f"rq_a{w}")
        eng.tensor_tensor(out=a, in0=y, in1=y, op=OP.mult)
        eng.tensor_tensor(out=a, in0=a, in1=in_ap, op=OP.mult)
        eng.tensor_scalar(
            out=a, in0=a, scalar1=-0.5, scalar2=1.5, op0=OP.mult, op1=OP.add)
        dst = out_ap if it == 1 else y
        eng.tensor_tensor(out=dst, in0=y, in1=a, op=OP.mult)


def _build_program():
    nc = bacc.Bacc(None)

    h_in = nc.declare_dram_parameter("h", [N, D], F32, isOutput=False)
    a16_in = nc.declare_dram_parameter("a16", [N, N], BF16, isOutput=False)
    bt_in = nc.declare_dram_parameter("bt", [HEADS, N, N], FP8, isOutput=False)
    wp32_in = nc.declare_dram_parameter("wp32", [P, 1666], F32, isOutput=False)
    wp16_in = nc.declare_dram_parameter("wp16", [P, 3464], BF16, isOutput=False)
    out_dram = nc.declare_dram_parameter("out", [N, D], F32, isOutput=True)
    taps = {}
    if DEBUG_TAPS:
        taps["t_disrow"] = nc.declare_dram_parameter(
            "t_disrow", [P, N], F32, isOutput=True)
        taps["t_GT"] = nc.declare_dram_parameter(
            "t_GT", [P, N], BF16, isOutput=True)
        taps["t_aT"] = nc.declare_dram_parameter(
            "t_aT", [P, NT, N], BF16, isOutput=True)
        taps["t_hnS"] = nc.declare_dram_parameter(
            "t_hnS", [P, NT, D], BF16, isOutput=True)
        taps["t_vna"] = nc.declare_dram_parameter(
            "t_vna", [P, NT, HEADS, DV + 1], BF16, isOutput=True)
        taps["t_et0"] = nc.declare_dram_parameter(
            "t_et0", [P, NT, N], BF16, isOutput=True)
        taps["t_mhcT"] = nc.declare_dram_parameter(
            "t_mhcT", [P, HEADS, N], BF16, isOutput=True)
        taps["t_orow"] = nc.declare_dram_parameter(
            "t_orow", [P, NT, D], F32, isOutput=True)
        taps["t_r2n"] = nc.declare_dram_parameter(
            "t_r2n", [P, NT, D], BF16, isOutput=True)
    disr = nc.dram_tensor("disr", [N], F32, kind="Internal")

    with tile.TileContext(nc) as tc, ExitStack() as ctx:
        consts = ctx.enter_context(tc.tile_pool(name="consts", bufs=1))
        persist = ctx.enter_context(tc.tile_pool(name="persist", bufs=1))
        small = ctx.enter_context(tc.tile_pool(name="small", bufs=12))

        # ---- constants -------------------------------------------------
        # diag-fix masks in bf16: omi = 1 - I, identb = I
        omi = consts.tile([P, P], BF16)
        nc.gpsimd.memset(omi, 1.0)
        nc.gpsimd.affine_select(
            out=omi, in_=omi, compare_op=OP.not_equal, fill=0.0,
            base=0, pattern=[[-1, P]], channel_multiplier=1)
        identb = consts.tile([P, P], BF16)
        nc.gpsimd.memset(identb, 0.0)
        nc.gpsimd.affine_select(
            out=identb, in_=identb, compare_op=OP.not_equal, fill=1.0,
            base=0, pattern=[[-1, P]], channel_multiplier=1)
        if USE_DOUBLEROW:
            # [I | 0] and [0 | I] fp8 stationaries for DoubleRow bias-adds
            id2a = consts.tile([P, 2, P], FP8)
            nc.gpsimd.memset(id2a, 0.0)
            nc.vector.tensor_copy(out=id2a[:, 0, :], in_=identb)
            id2b = consts.tile([P, 2, P], FP8)
            nc.gpsimd.memset(id2b, 0.0)
            nc.vector.tensor_copy(out=id2b[:, 1, :], in_=identb)
        else:
            id8 = consts.tile([P, P], FP8)
            nc.vector.tensor_copy(out=id8, in_=identb)
        ones1 = consts.tile([1, P], BF16)
        nc.vector.memset(ones1, 1.0)
        magic8 = consts.tile([P, NT], U32)
        nc.vector.memset(magic8, _RSQRT_MAGIC)
        one8 = consts.tile([P, NT], U32)
        nc.vector.memset(one8, 1)
        c158 = consts.tile([P, NT], F32)
        nc.vector.memset(c158, 1.5)
        cm058 = consts.tile([P, NT], F32)
        nc.vector.memset(cm058, -0.5)
        onescol = consts.tile([P, 1], BF16)
        nc.vector.memset(onescol, 1.0)

        # ---- phase A/B: H + A + packed-weight loads ---------------------
        h_sb = persist.tile([P, NT, D], F32, tag="h")
        nc.scalar.dma_start(out=h_sb, in_=h_in.rearrange("(t p) d -> p t d", p=P))
        wp32 = consts.tile([P, 1666], F32)
        nc.scalar.dma_start(out=wp32, in_=wp32_in[:, :])
        wp16 = consts.tile([P, 3464], BF16)
        nc.scalar.dma_start(out=wp16, in_=wp16_in[:, :])
        g0b = wp32[:, 0:128]
        be0b = wp32[:, 128:256]
        g2b = wp32[:, 256:384]
        g3b = wp32[:, 384:512]
        be23b = wp32[:, 512:640]
        bvb = wp32[:, 640:1664]
        b1p = wp32[:, 1664:1665]
        b2p = wp32[:, 1665:1666]
        mh_sb = wp16[:, 0:1024].rearrange("p (hh d) -> p hh d", hh=HEADS)
        wv_sb = wp16[:, 1024:2048]
        wo_sb = wp16[:, 2048:3072].rearrange("p (hh d) -> p hh d", hh=HEADS)
        w1_sb = wp16[:, 3072:3200]
        w2_sb = wp16[:, 3200:3328]
        qb_sb = wp16[:, 3328:3336]
        bo_row = wp16[0:1, 3336:3464]

        # A loads: natural chunks (scoped) feed DVE rowsums; A^T via XBAR
        # into 8 SEPARATE tiles (a shared tile serializes the XBARs against
        # the diag-fix RMWs through tile-level dep tracking).
        rs_all = small.tile([P, NT], F32, tag="rs_all")
        mvH = small.tile([P, NT, 2], F32, tag="mvH")
        anat_ctx = tc.tile_pool(name="anatp", bufs=1)
        anatp = anat_ctx.__enter__()
        anat = anatp.tile([P, NT, N], BF16, tag="anat")
        aTt = persist.tile([P, NT, N], BF16, tag="aTt")  # [m-chunk, n]
        aT = [aTt[:, i, :] for i in range(NT)]
        psT_ctx = tc.tile_pool(name="psT", bufs=4, space=bass.MemorySpace.PSUM)
        psT = psT_ctx.__enter__()
        rs_scr = anatp.tile([P, N], BF16, tag="rs_scr")
        for j in range(NT):
            an = anat[:, j, :]
            nc.sync.dma_start(out=an, in_=a16_in[j * P:(j + 1) * P, :])
            db = anat[:, j, j * P:(j + 1) * P]
            nc.gpsimd.tensor_tensor(out=db, in0=db, in1=omi, op=OP.mult)
            nc.gpsimd.tensor_tensor(out=db, in0=db, in1=identb, op=OP.add)
            if j % 2 == 0:
                nc.vector.reduce_sum(
                    out=rs_all[:, j:j + 1], in_=an, axis=AX.X)
            else:
                # rowsum on the ACT accumulator (scratch copy out)
                nc.scalar.activation(
                    out=rs_scr, in_=an, func=AF.Copy,
                    accum_out=rs_all[:, j:j + 1])
            # transpose the 8 blocks of this natural chunk on the (idle) PE;
            # one strided 512-wide copy per 4 blocks, alternating ACT/DVE
            for g in range(2):
                pt = psT.tile([P, 4, P], BF16, tag="pt", name=f"pt{j}_{g}")
                for ii in range(4):
                    i = g * 4 + ii
                    nc.tensor.transpose(
                        pt[:, ii, :], anat[:, j, i * P:(i + 1) * P], identb)
                dst = aTt[:, g * 4:(g + 1) * 4, j * P:(j + 1) * P]
                nc.scalar.copy(out=dst, in_=pt)
        psT_ctx.__exit__(None, None, None)
        for j in range(NT):
            # H LN stats (independent of A)
            s6 = small.tile([P, 6], F32, tag="s6h", name=f"s6h{j}")
            nc.vector.bn_stats(out=s6, in_=h_sb[:, j, :])
            nc.vector.bn_aggr(out=mvH[:, j, :], in_=s6)

        # dis = rsqrt(max(rowsum, 1))  [P, NT]
        dmax = small.tile([P, NT], F32, tag="dmax")
        nc.vector.tensor_scalar_max(out=dmax, in0=rs_all, scalar1=1.0)
        dis_sb = small.tile([P, NT], F32, tag="dis")
        _rsqrt(nc, nc.vector, small, dis_sb, dmax, NT, "dis", magic8)
        anat_ctx.__exit__(None, None, None)

        # HnS = dis * LN(H):  rstd2 = rstdH*dis
        tvH = small.tile([P, NT], F32, tag="tvH")
        nc.vector.tensor_scalar_add(
            out=tvH, in0=mvH[:, :, 1], scalar1=EPS)
        rstdH = small.tile([P, NT], F32, tag="rstdH")
        _rsqrt(nc, nc.vector, small, rstdH, tvH, NT, "H", magic8)
        rstd2 = small.tile([P, NT], F32, tag="rstd2")
        nc.vector.tensor_tensor(out=rstd2, in0=rstdH, in1=dis_sb, op=OP.mult)

        # disrow [P, N] broadcast of dis in n-order via DRAM round-trip
        nc.sync.dma_start(out=disr.rearrange("(t p) -> p t", p=P), in_=dis_sb)
        disrow = persist.tile([P, N], F32, tag="disrow")
        rep = bass.AP(
            tensor=disr[:].tensor, offset=disr[:].offset,
            ap=[[0, P]] + list(disr[:].ap))
        nc.sync.dma_start(out=disrow, in_=rep)

        hnS = persist.tile([P, NT, D], BF16, tag="hnS")
        for j in range(NT):
            zh = small.tile([P, D], F32, tag="zh")
            nc.vector.tensor_scalar(
                out=zh, in0=h_sb[:, j, :], scalar1=mvH[:, j, 0:1],
                scalar2=rstd2[:, j:j + 1], op0=OP.subtract, op1=OP.mult)
            hn1 = small.tile([P, D], BF16, tag="hn1")
            nc.gpsimd.tensor_tensor(out=hn1, in0=zh, in1=g0b, op=OP.mult)
            be0S = small.tile([P, D], BF16, tag="be0S")
            nc.gpsimd.tensor_scalar_mul(
                out=be0S, in0=be0b, scalar1=dis_sb[:, j:j + 1])
            nc.gpsimd.tensor_tensor(
                out=hnS[:, j, :], in0=hn1, in1=be0S, op=OP.add)

        # ---- G^T = dis_n * (HnS^T @ Ahat^T)  [d, n] bf16 ---------------
        GT = persist.tile([P, N], BF16, tag="GT")
        with tc.tile_pool(name="psG", bufs=2, space=bass.MemorySpace.PSUM) as psG:
            for c in range(2):
                g0t = psG.tile([P, 512], F32, tag="g0t")
                for j in range(NT):
                    nc.tensor.matmul(
                        g0t, hnS[:, j, :], aT[j][:, c * 512:(c + 1) * 512],
                        start=(j == 0), stop=(j == NT - 1))
                nc.vector.tensor_tensor(
                    out=GT[:, c * 512:(c + 1) * 512], in0=g0t,
                    in1=disrow[:, c * 512:(c + 1) * 512], op=OP.mult)

        if DEBUG_TAPS:
            nc.sync.dma_start(out=taps["t_disrow"][:, :], in_=disrow)
            nc.sync.dma_start(out=taps["t_GT"][:, :], in_=GT)
            for _i in range(NT):
                nc.sync.dma_start(out=taps["t_aT"][:, _i, :], in_=aT[_i])
            nc.sync.dma_start(out=taps["t_hnS"][:, :, :], in_=hnS)

        # ---- V = GT^T Wv + bv (vna with ones column), v = GT^T qb ------
        vna = persist.tile([P, NT, HEADS, DV], BF16, tag="vna")
        v_sb = persist.tile([P, NT, HEADS], F32, tag="v_sb")
        with tc.tile_pool(name="psV", bufs=2, space=bass.MemorySpace.PSUM) as psV, \
             tc.tile_pool(name="psv", bufs=2, space=bass.MemorySpace.PSUM) as psv:
            for i in range(NT):
                vq = psv.tile([P, HEADS], F32, tag="vq")
                nc.tensor.matmul(
                    vq, GT[:, i * P:(i + 1) * P], qb_sb, start=True, stop=True)
                nc.vector.tensor_copy(out=v_sb[:, i, :], in_=vq)
                for c in range(2):
                    vp = psV.tile([P, 512], F32, tag="vp")
                    nc.tensor.matmul(
                        vp, GT[:, i * P:(i + 1) * P],
                        wv_sb[:, c * 512:(c + 1) * 512], start=True, stop=True)
                    nc.vector.tensor_tensor(
                        out=vna[:, i, c * 4:(c + 1) * 4, 0:DV],
                        in0=vp.rearrange("p (a b) -> p a b", a=4),
                        in1=bvb[:, c * 512:(c + 1) * 512].rearrange(
                            "p (a b) -> p a b", a=4),
                        op=OP.add)

        # ---- phase D: attention, software-pipelined over heads ---------
        # PE stream per head: [P_h matmuls] -> [PV of head h-1] -> [S+B of h],
        # so the tensor engine never waits for the ACT exp chain.
        mhcT = persist.tile([P, HEADS, N], BF16, tag="mhcT")
        bt_tiles = {}
        et_tiles = {}
        psb_tiles = {}
        BFLAT = NT * N + 512  # flat per-head B^T + zero pad for DoubleRow view
        with tc.tile_pool(name="btp", bufs=3) as btp, \
             tc.tile_pool(name="pp", bufs=2) as ppool, \
             tc.tile_pool(name="etp", bufs=2) as etp, \
             tc.tile_pool(name="zrp", bufs=6) as zrp, \
             tc.tile_pool(name="psS", bufs=3, space=bass.MemorySpace.PSUM) as psS, \
             tc.tile_pool(name="psM", bufs=2, space=bass.MemorySpace.PSUM) as psM:

            def load_bt(h):
                t = btp.tile([P, BFLAT], FP8, tag="bt", name=f"bt{h}")
                nc.gpsimd.memset(t[:, NT * N:], 0.0)
                nc.sync.dma_start(
                    out=t[:, 0:NT * N].rearrange("p (t n) -> p t n", t=NT),
                    in_=bt_in[h].rearrange("(t p) n -> p t n", p=P))
                bt_tiles[h] = t

            def prep_head(h):
                pp = psS.tile([P, N], F32, tag="st", name=f"pp{h}")
                for c in range(2):
                    nc.tensor.matmul(
                        pp[:, c * 512:(c + 1) * 512], mh_sb[:, h, :],
                        GT[:, c * 512:(c + 1) * 512], start=True, stop=True)
                p_sb = ppool.tile([P, N], BF16, tag="p_sb", name=f"psb{h}")
                for c in range(2):
                    # head 0/1 copies on ACT (idle at startup; DVE is busy
                    # with the V-phase adds then)
                    if h < 2:
                        nc.scalar.copy(
                            out=p_sb[:, c * 512:(c + 1) * 512],
                            in_=pp[:, c * 512:(c + 1) * 512])
                    else:
                        nc.vector.tensor_copy(
                            out=p_sb[:, c * 512:(c + 1) * 512],
                            in_=pp[:, c * 512:(c + 1) * 512])
                psb_tiles[h] = p_sb

            def s_loop(h, j0, j1):
                if j0 == 0:
                    et_tiles[h] = etp.tile(
                        [P, NT, N], BF16, tag="et", name=f"et{h}")
                et = et_tiles[h]
                btt = bt_tiles[h]
                p_sb = psb_tiles[h]
                for j in range(j0, j1):
                    st = psS.tile([P, N], F32, tag="st", name=f"st{h}_{j}")
                    for c in range(2):
                        nc.tensor.matmul(
                            st[:, c * 512:(c + 1) * 512],
                            p_sb[:, j * P:(j + 1) * P],
                            GT[:, c * 512:(c + 1) * 512],
                            start=True, stop=False)
                    for c in range(2):
                        if USE_DOUBLEROW:
                            rv = btt[:, j * N + c * 512:
                                     j * N + c * 512 + 1024].rearrange(
                                "p (a b) -> p a b", a=2)
                            nc.tensor.matmul(
                                st[:, c * 512:(c + 1) * 512], id2a, rv,
                                start=False, stop=True,
                                perf_mode=PM.DoubleRow, skip_group_check=True)
                        else:
                            nc.tensor.matmul(
                                st[:, c * 512:(c + 1) * 512], id8,
                                btt[:, j * N + c * 512:j * N + (c + 1) * 512],
                                start=False, stop=True, skip_group_check=True)
                    nc.scalar.activation(
                        out=et[:, j, :], in_=st, func=AF.Exp,
                        bias=v_sb[:, j, h:h + 1])
                if DEBUG_TAPS and h == 0:
                    nc.sync.dma_start(out=taps["t_et0"][:, :, :], in_=et)

            def pv_half(h, half):
                et = et_tiles[h]
                pm = psM.tile([P, 4, P], F32, tag="pm", name=f"pm{h}_{half}")
                mvv = small.tile([P, 4, 2], F32, tag="mvv",
                                 name=f"mvv{h}_{half}")
                for q in range(4):
                    i = half * 4 + q
                    for j in range(NT):
                        nc.tensor.matmul(
                            pm[:, q, :],
                            et[:, j, i * P:(i + 1) * P],
                            vna[:, j, h, :],
                            start=(j == 0), stop=(j == NT - 1))
                    s6 = small.tile([P, 6], F32, tag="s6a",
                                    name=f"s6a{h}_{half}_{q}")
                    nc.vector.bn_stats(out=s6, in_=pm[:, q, :])
                    nc.vector.bn_aggr(out=mvv[:, q, :], in_=s6)
                t3 = small.tile([P, 4], F32, tag="t3", name=f"t3{h}_{half}")
                nc.vector.tensor_scalar_add(
                    out=t3, in0=mvv[:, :, 1], scalar1=EPS)
                rstd = small.tile([P, 4], F32, tag="rstda",
                                  name=f"rsd{h}_{half}")
                _rsqrt_pool(nc, small, rstd, t3, 4, f"a{h}_{half}", {
                    "one": one8[:, 0:4], "magic": magic8[:, 0:4],
                    "c15": c158[:, 0:4], "cm05": cm058[:, 0:4]})
                for q in range(4):
                    i = half * 4 + q
                    zrow = zrp.tile([P, DV], BF16, tag="zrow",
                                    name=f"z{h}_{half}_{q}")
                    nc.vector.tensor_scalar(
                        out=zrow, in0=pm[:, q, :],
                        scalar1=mvv[:, q, 0:1],
                        scalar2=rstd[:, q:q + 1],
                        op0=OP.subtract, op1=OP.mult)
                    nc.sync.dma_start_transpose(
                        out=mhcT[:, h, i * P:(i + 1) * P], in_=zrow)

            load_bt(0)
            load_bt(1)
            prep_head(0)
            prep_head(1)
            s_loop(0, 0, NT)
            for h in range(1, HEADS):
                if h + 1 < HEADS:
                    load_bt(h + 1)
                s_loop(h, 0, 2)
                pv_half(h - 1, 0)
                s_loop(h, 2, 4)
                if h + 1 < HEADS:
                    prep_head(h + 1)
                pv_half(h - 1, 1)
                s_loop(h, 4, NT)
            pv_half(HEADS - 1, 0)
            pv_half(HEADS - 1, 1)

        # ---- phase E: output projection + MLP --------------------------
        # WO chunks 0-3 only need pv_half(7, 0); they overlap the final half.
        orow_sb = persist.tile([P, NT, D], F32, tag="orow")
        mvO = small.tile([P, NT, 2], F32, tag="mvO")
        with tc.tile_pool(name="psE", bufs=2, space=bass.MemorySpace.PSUM) as psE:
            def wo_chunk(i):
                op = psE.tile([P, D], F32, tag="op", name=f"op{i}")
                for hh in range(HEADS):
                    nc.tensor.matmul(
                        op, mhcT[:, hh, i * P:(i + 1) * P], wo_sb[:, hh, :],
                        start=(hh == 0), stop=False)
                nc.tensor.matmul(
                    op, ones1, bo_row, start=False, stop=True,
                    skip_group_check=True)
                nc.vector.tensor_tensor(
                    out=orow_sb[:, i, :], in0=op, in1=h_sb[:, i, :], op=OP.add)
                s6 = small.tile([P, 6], F32, tag="s6o", name=f"s6o{i}")
                nc.vector.bn_stats(out=s6, in_=orow_sb[:, i, :])
                nc.vector.bn_aggr(out=mvO[:, i, :], in_=s6)
            for i in range(NT):
                wo_chunk(i)
        if DEBUG_TAPS:
            nc.sync.dma_start(out=taps["t_orow"][:, :, :], in_=orow_sb)
        tvO = small.tile([P, NT], F32, tag="tvO")
        nc.vector.tensor_scalar_add(
            out=tvO, in0=mvO[:, :, 1], scalar1=EPS)
        rstdO = small.tile([P, NT], F32, tag="rstdO")
        _rsqrt(nc, nc.vector, small, rstdO, tvO, NT, "O", magic8)
        oz = persist.tile([P, NT, D], BF16, tag="oz")
        oT = persist.tile([P, N], BF16, tag="oT")
        f1_sb = persist.tile([P, NT, D], F32, tag="f1_sb")
        psE1_ctx = tc.tile_pool(name="psE1", bufs=2, space=bass.MemorySpace.PSUM)
        psE1 = psE1_ctx.__enter__()
        for g in range(2):
            pt = psE1.tile([P, 4, P], BF16, tag="eT", name=f"ozT{g}")
            for ii in range(4):
                i = g * 4 + ii
                nc.vector.tensor_scalar(
                    out=oz[:, i, :], in0=orow_sb[:, i, :],
                    scalar1=mvO[:, i, 0:1],
                    scalar2=rstdO[:, i:i + 1], op0=OP.subtract, op1=OP.mult)
                nc.tensor.transpose(pt[:, ii, :], oz[:, i, :], identb)
                # f1 = oz*g2 + be23 computed early on Pool, overlapping the MLP
                nc.gpsimd.tensor_tensor(
                    out=f1_sb[:, i, :], in0=oz[:, i, :], in1=g2b, op=OP.mult)
                nc.gpsimd.tensor_tensor(
                    out=f1_sb[:, i, :], in0=f1_sb[:, i, :], in1=be23b,
                    op=OP.add)
            nc.scalar.copy(
                out=oT[:, g * 512:(g + 1) * 512],
                in_=pt.rearrange("p a b -> p (a b)"))

        r1T = persist.tile([P, N], BF16, tag="r1T")
        r2T = persist.tile([P, N], BF16, tag="r2T")
        with tc.tile_pool(name="psE2", bufs=2, space=bass.MemorySpace.PSUM) as psE2:
            for c in range(2):
                ps = psE2.tile([P, 512], F32, tag="ps2")
                nc.tensor.matmul(
                    ps, w1_sb, oT[:, c * 512:(c + 1) * 512], start=True, stop=True)
                nc.scalar.activation(
                    out=r1T[:, c * 512:(c + 1) * 512], in_=ps, func=AF.Relu,
                    bias=b1p)
            for c in range(2):
                ps = psE2.tile([P, 512], F32, tag="ps2")
                nc.tensor.matmul(
                    ps, w2_sb, r1T[:, c * 512:(c + 1) * 512], start=True, stop=True)
                nc.scalar.activation(
                    out=r2T[:, c * 512:(c + 1) * 512], in_=ps, func=AF.Relu,
                    bias=b2p)

        mvR = small.tile([P, NT, 2], F32, tag="mvR")
        r2p = []
        for g in range(2):
            pt = psE1.tile([P, 4, P], BF16, tag="eT", name=f"r2T{g}")
            r2p.append(pt)
            for ii in range(4):
                i = g * 4 + ii
                nc.tensor.transpose(
                    pt[:, ii, :], r2T[:, i * P:(i + 1) * P], identb)
                s6 = small.tile([P, 6], F32, tag="s6r", name=f"s6r{i}")
                nc.vector.bn_stats(out=s6, in_=pt[:, ii, :])
                nc.vector.bn_aggr(out=mvR[:, i, :], in_=s6)
        if DEBUG_TAPS:
            for _g in range(2):
                nc.sync.dma_start(
                    out=taps["t_r2n"][:, _g * 4:(_g + 1) * 4, :], in_=r2p[_g])
        tvR = small.tile([P, NT], F32, tag="tvR")
        nc.vector.tensor_scalar_add(
            out=tvR, in0=mvR[:, :, 1], scalar1=EPS)
        rstdR = small.tile([P, NT], F32, tag="rstdR")
        _rsqrt(nc, nc.vector, small, rstdR, tvR, NT, "R", magic8)

        out_sb = persist.tile([P, NT, D], F32, tag="osb")
        for i in range(NT):
            zr = small.tile([P, D], F32, tag="zr", name=f"zr{i}")
            nc.vector.tensor_scalar(
                out=zr, in0=r2p[i // 4][:, i % 4, :], scalar1=mvR[:, i, 0:1],
                scalar2=rstdR[:, i:i + 1], op0=OP.subtract, op1=OP.mult)
            f2 = small.tile([P, D], F32, tag="f2", name=f"f2{i}")
            nc.vector.tensor_tensor(out=f2, in0=zr, in1=g3b, op=OP.mult)
            nc.vector.tensor_tensor(
                out=out_sb[:, i, :], in0=f1_sb[:, i, :], in1=f2, op=OP.add)
        psE1_ctx.__exit__(None, None, None)
        for g in range(2):
            nc.sync.dma_start(
                out=out_dram.rearrange(
                    "(t p) d -> p t d", p=P)[:, g * 4:(g + 1) * 4, :],
                in_=out_sb[:, g * 4:(g + 1) * 4, :])

    nc.compile()
    return nc


def _get_program():
    if "nc" not in _prog_cache:
        _prog_cache["nc"] = _build_program()
    return _prog_cache["nc"]


def kernel(**inputs):
    nc = _get_program()
    f32 = np.float32
    bf16 = ml_dtypes.bfloat16
    fp8 = ml_dtypes.float8_e4m3

    H = np.asarray(inputs["H"], dtype=f32)
    A = np.asarray(inputs["A"], dtype=f32)
    WQ = np.asarray(inputs["W_Q"], dtype=f32)
    WK = np.asarray(inputs["W_K"], dtype=f32)
    WV = np.asarray(inputs["W_V"], dtype=f32)
    WO = np.asarray(inputs["W_O"], dtype=f32)
    bQ = np.asarray(inputs["b_Q"], dtype=f32)
    g1 = np.asarray(inputs["g1"], dtype=f32)
    be1 = np.asarray(inputs["be1"], dtype=f32)
    g2 = np.asarray(inputs["g2"], dtype=f32)
    be2 = np.asarray(inputs["be2"], dtype=f32)
    W1 = np.asarray(inputs["W1"], dtype=f32)
    b1 = np.asarray(inputs["b1"], dtype=f32)

    mh = np.stack([
        DK * WK[:, h * DV:(h + 1) * DV] @ WQ[:, h * DV:(h + 1) * DV].T
        for h in range(HEADS)
    ]).astype(bf16)
    qb = np.stack([
        DK * WK[:, h * DV:(h + 1) * DV] @ bQ[h * DV:(h + 1) * DV]
        for h in range(HEADS)
    ], axis=1).astype(bf16)  # [D, HEADS]
    WOp = (g1[:, None] * WO.reshape(HEADS, DV, D)).astype(bf16)  # [H, DV, D]
    bo = (be1 @ WO.reshape(HEADS, DV, D).sum(0)).reshape(1, D).astype(bf16)
    W1p = (g2[:, None] * W1).astype(bf16)
    b1p = (be2 @ W1 + b1).reshape(D, 1).astype(f32)
    be23 = (be2 + np.asarray(inputs["be3"], dtype=f32)).astype(f32)

    BT = np.ascontiguousarray(
        np.asarray(inputs["B_bias"], dtype=f32).transpose(0, 2, 1)).astype(fp8)

    wp32 = np.zeros((P, 1666), f32)
    wp32[:, 0:128] = np.asarray(inputs["g0"], dtype=f32)[None, :]
    wp32[:, 128:256] = np.asarray(inputs["be0"], dtype=f32)[None, :]
    wp32[:, 256:384] = g2[None, :]
    wp32[:, 384:512] = np.asarray(inputs["g3"], dtype=f32)[None, :]
    wp32[:, 512:640] = be23[None, :]
    wp32[:, 640:1664] = np.asarray(inputs["b_V"], dtype=f32)[None, :]
    wp32[:, 1664] = b1p[:, 0]
    wp32[:, 1665] = np.asarray(inputs["b2"], dtype=f32)
    wp16 = np.zeros((P, 3464), bf16)
    wp16[:, 0:1024] = mh.transpose(1, 0, 2).reshape(P, HEADS * D)
    wp16[:, 1024:2048] = WV.astype(bf16)
    wp16[:, 2048:3072] = WOp.transpose(1, 0, 2).reshape(P, HEADS * D)
    wp16[:, 3072:3200] = W1p
    wp16[:, 3200:3328] = np.asarray(inputs["W2"], dtype=f32).astype(bf16)
    wp16[:, 3328:3336] = qb
    wp16[:, 3336:3464] = bo[None, 0, :]
    base = {
        "bt": BT,
        "wp32": wp32,
        "wp16": np.ascontiguousarray(wp16),
    }

    in_maps = []
    for c in range(B):
        m = dict(base)
        m["h"] = np.ascontiguousarray(H[c])
        m["a16"] = np.ascontiguousarray(A[c]).astype(bf16)
        in_maps.append(m)

    res = run_bass_kernel_spmd(nc, in_maps, list(range(B)))
    if DEBUG_TAPS:
        _prog_cache["taps"] = res.results
    out = np.stack([res.results[c]["out"] for c in range(B)], axis=0)
    return out.astype(np.float32)


if __name__ == "__main__":
    _get_program()
    print("program built ok")
